# revision 22
# baseline (speedup 1.0000x reference)
"""DualTransformerBlock Trainium2 kernel (v2).

Strategy (8 cores: core c -> sample b=c//2, token half h=c%2, T=2048):
  - EfficientAttention reassociated: att = V @ ((K^T Q)/colsum @ wr.T); the
    [N,N] context never materializes.  The K/Q-side stats (exp projections and
    the [C,C] S matrix) are computed REDUNDANTLY for all 4096 tokens on both
    cores of a pair, so no collective is needed for EA.
  - Each core's x tensors are ordered with ITS half first (tiles 0..15 = own
    half, 16..31 = peer half); the S/stat sums are order-invariant, so one
    SPMD program serves all cores.
  - ChannelAttention still needs one cross-half reduction (per-head gram +
    q/k norms); exchanged via AllGather (cheaper than AllReduce in the cost
    model) and summed locally.
  - LayerNorm: mean-centering is folded into the following weight matrices on
    the host ((x-mean) @ W == x @ (Pc W)); the per-token 1/sigma comes from a
    bit-trick Newton rsqrt (no activation-table use) and is fused into Act
    scale= or evacuation scalars.
  - Heavy matmuls run in fp8e4m3 with DoubleRow perf mode (2 contraction
    tiles per instruction, 0.5 cyc/row).  Host-side exponent scaling keeps
    fp8 operands out of the subnormal range; scales cancel or fold into
    per-token evacuation scalars.
  - Activation tables: only Exp (EA) and Gelu (MLPs) -> 2 table loads.  The
    tiny per-head channel-attn softmax uses a polynomial exp on DVE/Pool.
"""

import os
import sys

sys.path.insert(0, "/opt/trn_rl_repo")

import numpy as np
import ml_dtypes

import concourse.bass as bass
import concourse.mybir as mybir
from concourse import bacc
from concourse.tile import TileContext

F32 = mybir.dt.float32
BF16 = mybir.dt.bfloat16
F8 = mybir.dt.float8e4
I32 = mybir.dt.int32
AF = mybir.ActivationFunctionType
OP = mybir.AluOpType
AX = mybir.AxisListType
DR = mybir.MatmulPerfMode.DoubleRow

B, N, C = 4, 4096, 256
H_CH = 8
HD = C // H_CH          # 32
DFF = 4 * C             # 1024
EPS_LN = 1e-5

NCORES = 8
T = N // 2              # 2048 tokens per core half
NT = N // 128           # 32 token tiles (full sample)
NTH = T // 128          # 16 token tiles (own half)
CT = C // 128           # 2 channel tiles
FT = DFF // 128         # 8 ff tiles
REPLICA_GROUPS = [[0, 1], [2, 3], [4, 5], [6, 7]]
CCW = 2 * HD + 2 * CT   # collective payload width (68)

# fp8 exponent scales (host folds these into weights; device descales)
SW = 32.0       # generic weight scale (wkq, wv, w1..w4, qkv-v, proj)
SQK = 8.0       # channel-attn qk scale
SK8 = 64.0      # normalized-k softmax scale
SWR = 256.0     # wr scale
MAGIC = 0x5F3759DF

_CACHE = {}


def build_program():
    if "nc" in _CACHE:
        return _CACHE["nc"]
    nc = bacc.Bacc(None, target_bir_lowering=False)

    io = {}

    def param(name, shape, dt=F32):
        io[name] = nc.declare_dram_parameter(name, list(shape), dt, isOutput=False)

    param("xt16", (128, NT, C), BF16)       # token-major x, all tokens, own half first
    param("xcm8", (128, CT, N), F8)         # channel-major x, all tokens
    param("wkq8", (128, CT, 2 * C), F8)     # [K|Q] proj  (centered, g1, xSW)
    param("wv8", (128, CT, C), F8)          # EA values proj (centered, g1, xSW)
    param("wr16", (128, CT, C), BF16)       # EA out proj (xSWR)
    param("qk8", (128, CT, 2 * C), F8)      # CA [q|k] proj (centered, g3, xSQK)
    param("vv8", (128, CT, C), F8)          # CA v proj (centered, g3, xSW)
    param("pj8", (128, CT, C), F8)          # CA out proj (xSW)
    param("w1_8", (128, CT, DFF), F8)       # MLP1 fc1 (centered, g2, xSW)
    param("w2_8", (128, FT, C), F8)         # MLP1 fc2 (xSW)
    param("w3_8", (128, CT, DFF), F8)       # MLP2 fc1 (centered, g4, xSW)
    param("w4_8", (128, FT, C), F8)         # MLP2 fc2 (xSW)
    param("ident8", (128, 128), F8)
    param("ident16", (128, 128), BF16)
    param("ident32", (128, 128), F32)
    param("ones8", (128, 1), F8)
    param("bdmask", (128, CT, 128), BF16)   # per-head 32x32 block-diag mask
    param("temp_c", (128, CT), F32)         # CA temperature, column layout
    io["y"] = nc.declare_dram_parameter("y", [T, C], F32, isOutput=True)
    DBG = os.environ.get("KDBG", "0") == "1"
    dbg_specs = {
        "d_rs1": (128, NT), "d_kq": (128, 2 * C), "d_st": (128, CT, C),
        "d_csum": (128, CT), "d_s2": (128, CT, C), "d_add1": (128, C),
        "d_vcm": (128, CT, 512), "d_add2": (128, C), "d_qkt": (128, 2 * C),
        "d_catot": (128, CCW), "d_attn": (128, CT, 128), "d_add3": (128, C),
        "d_rs2": (128, NTH),
    }
    if DBG:
        for k, shp in dbg_specs.items():
            io[k] = nc.declare_dram_parameter(k, list(shp), F32, isOutput=True)

    cc_in = nc.dram_tensor("cc_in", [128, CCW], F32)
    cc_out = nc.dram_tensor("cc_out", [2, 128, CCW], F32)

    with TileContext(nc) as tc:
        with (
            tc.tile_pool(name="wpool", bufs=1) as wp,
            tc.tile_pool(name="apool", bufs=1) as ap,
            tc.tile_pool(name="tmp", bufs=3) as tp,
            tc.tile_pool(name="stage", bufs=1) as stg,
            tc.tile_pool(name="pacc", bufs=1, space="PSUM") as pacc,
            tc.tile_pool(name="pmm", bufs=3, space="PSUM") as pmm,
            tc.tile_pool(name="pmmt", bufs=2, space="PSUM") as pmmt,
        ):
            # ---------------- input DMA ----------------
            xt = ap.tile([128, NT, C], BF16, tag="xt")
            for g in range(4):
                nc.sync.dma_start(out=xt[:, g * 8:(g + 1) * 8, :],
                                  in_=io["xt16"][:, g * 8:(g + 1) * 8, :])
            xcm = ap.tile([128, CT, N], F8, tag="xcm")
            for g in range(4):
                nc.sync.dma_start(
                    out=xcm[:, :, g * (N // 4):(g + 1) * (N // 4)],
                    in_=io["xcm8"][:, :, g * (N // 4):(g + 1) * (N // 4)])

            def wload(name, d1, d2, dt=F8, tag=None):
                t = wp.tile([128, d1, d2], dt, tag=tag or name)
                nc.sync.dma_start(out=t, in_=io[name][:, :, :])
                return t

            wkq = wload("wkq8", CT, 2 * C)
            wv = wload("wv8", CT, C)
            wr16 = wload("wr16", CT, C, BF16)
            w1 = wload("w1_8", CT, DFF, tag="wmlp_a")
            w2 = wload("w2_8", FT, C, tag="wmlp_b")
            ident8 = wp.tile([128, 128], F8, tag="ident8")
            nc.sync.dma_start(out=ident8, in_=io["ident8"][:, :])
            ident16 = wp.tile([128, 128], BF16, tag="ident16")
            nc.sync.dma_start(out=ident16, in_=io["ident16"][:, :])
            ident32 = wp.tile([128, 128], F32, tag="ident32")
            nc.sync.dma_start(out=ident32, in_=io["ident32"][:, :])
            ones8 = wp.tile([128, 1], F8, tag="ones8")
            nc.sync.dma_start(out=ones8, in_=io["ones8"][:, :])

            def dump(name, src_ap):
                if DBG:
                    dt = stg.tile(list(src_ap.shape), F32, tag=f"dbg_{name}",
                                  name=f"dbg_{name}", bufs=1)
                    nc.vector.tensor_copy(dt, src_ap)
                    nc.sync.dma_start(out=io[name].__getitem__(
                        tuple([slice(None)] * len(src_ap.shape))), in_=dt)

            # ---------------- helpers ----------------
            def ln_chunk(src_t, tiles, y, c0, m, tag, div=1.0):
                """Write rsqrt(var+eps)/div for tiles[c0:c0+m] into y[:, c0:c0+m]."""
                mvg = tp.tile([128, m, 2], F32, tag=f"mvg_{tag}",
                              name=f"mvg_{tag}_{c0}", bufs=2)
                for j in range(m):
                    st6 = tp.tile([128, 6], BF16, tag="st6", bufs=4)
                    nc.vector.bn_stats(out=st6, in_=src_t[:, tiles[c0 + j], :])
                    nc.vector.bn_aggr(out=mvg[:, j, :], in_=st6)
                vpe = tp.tile([128, m], F32, tag=f"vpe_{tag}",
                              name=f"vpe_{tag}_{c0}", bufs=2)
                nc.vector.tensor_scalar(out=vpe, in0=mvg[:, :, 1],
                                        scalar1=EPS_LN, scalar2=None,
                                        op0=OP.add)
                ys = y[:, c0:c0 + m]
                nc.vector.tensor_scalar(out=ys.bitcast(I32),
                                        in0=vpe.bitcast(I32),
                                        scalar1=1, scalar2=None,
                                        op0=OP.logical_shift_right)
                nc.vector.tensor_scalar(out=ys.bitcast(I32),
                                        in0=ys.bitcast(I32),
                                        scalar1=-1, scalar2=MAGIC,
                                        op0=OP.mult, op1=OP.add)
                t_ = tp.tile([128, m], F32, tag=f"nt_{tag}",
                             name=f"nt_{tag}_{c0}", bufs=2)
                for it in range(2):
                    last = it == 1
                    nc.gpsimd.tensor_tensor(out=t_, in0=ys, in1=ys, op=OP.mult)
                    nc.gpsimd.tensor_tensor(out=t_, in0=t_, in1=vpe, op=OP.mult)
                    nc.vector.tensor_scalar(
                        out=t_, in0=t_,
                        scalar1=(-0.5 / div) if last else -0.5,
                        scalar2=(1.5 / div) if last else 1.5,
                        op0=OP.mult, op1=OP.add)
                    nc.gpsimd.tensor_tensor(out=ys, in0=ys, in1=t_, op=OP.mult)

            def ln_rs(src_t, tiles, tag, div=1.0, chunk=8):
                n = len(tiles)
                y = stg.tile([128, n], F32, tag=f"rs_{tag}", name=f"rs_{tag}")
                for c0 in range(0, n, chunk):
                    ln_chunk(src_t, tiles, y, c0, min(chunk, n - c0), tag, div)
                return y

            def bscale(sc, j0, n, width=C):
                """[128, n] slice of sc -> stride-0 broadcast [128, n, width]."""
                return sc[:, j0:j0 + n].unsqueeze(-1).broadcast_to((128, n, width))

            def cm_group(src_t, rs, x16, cm, g, tag):
                """tiles 4g..4g+4: mult (alt Pool/DVE) + DMA xbar transposes."""
                with nc.allow_low_precision(reason="bf16 matmul operand"):
                    eng = nc.gpsimd if g % 2 == 0 else nc.vector
                    eng.tensor_tensor(
                        out=x16[:, g * 4:(g + 1) * 4, :],
                        in0=src_t[:, g * 4:(g + 1) * 4, :],
                        in1=bscale(rs, g * 4, 4), op=OP.mult)
                    for t in range(g * 4, g * 4 + 4):
                        nc.sync.dma_start_transpose(
                            cm[:, :, t * 128:(t + 1) * 128], x16[:, t, :])

            def cm_tiles(tag):
                x16 = ap.tile([128, NTH, C], BF16, tag=f"tm_{tag}",
                              name=f"tm_{tag}")
                cm = ap.tile([128, CT, T], BF16, tag=f"cm_{tag}",
                             name=f"cm_{tag}")
                return x16, cm

            # ================= EfficientAttention =================
            # ================= EfficientAttention =================
            # ================= EfficientAttention =================
            # sc_kq = rs1/SW for ALL 32 tiles (stats replicated across pair);
            # LN1 chunks are interleaved into the projection loop below.
            sc_kq = stg.tile([128, NT], F32, tag="rs_ln1")

            kq8 = ap.tile([128, NT, 2 * C], F8, tag="kq8")
            ksums = stg.tile([128, NT], F32, tag="ksums")
            rinv64 = stg.tile([128, NT], F32, tag="rinv64")
            ps_s0 = pacc.tile([128, C], F32, tag="ps_s0")
            ps_s1 = pacc.tile([128, C], F32, tag="ps_s1")

            for p in range(NT // 2):
                if p % 4 == 0:
                    ln_chunk(xt, list(range(NT)), sc_kq, p * 2, 8, "ln1", div=SW)
                for i in range(2):
                    t = 2 * p + i
                    ps = pmm.tile([128, 2 * C], F32, tag="mm")
                    nc.tensor.matmul(ps, xcm[:, :, t * 128:(t + 1) * 128],
                                     wkq, start=True, stop=True, perf_mode=DR)
                    with nc.allow_low_precision(reason="fp8 exp"):
                        nc.scalar.activation(kq8[:, t, :], ps, AF.Exp,
                                             scale=sc_kq[:, t:t + 1])
                # ksum + SK8/ksum for the pair
                nc.vector.tensor_reduce(
                    ksums[:, 2 * p:2 * p + 2].unsqueeze(-1),
                    kq8[:, 2 * p:2 * p + 2, 0:C], axis=AX.X, op=OP.add)
                nc.vector.reciprocal(rinv64[:, 2 * p:2 * p + 2],
                                     ksums[:, 2 * p:2 * p + 2])
                nc.vector.tensor_scalar(out=rinv64[:, 2 * p:2 * p + 2],
                                        in0=rinv64[:, 2 * p:2 * p + 2],
                                        scalar1=SK8, scalar2=None, op0=OP.mult)
                with nc.allow_low_precision(reason="fp8 softmax-k scale"):
                    keng = nc.vector if p % 3 == 0 else nc.gpsimd
                    keng.tensor_tensor(
                        out=kq8[:, 2 * p:2 * p + 2, 0:C],
                        in0=kq8[:, 2 * p:2 * p + 2, 0:C],
                        in1=bscale(rinv64, 2 * p, 2), op=OP.mult)
                # S accumulation (DoubleRow over the token-tile pair)
                st_, sp_ = (p == 0), (p == NT // 2 - 1)
                nc.tensor.matmul(ps_s0, kq8[:, 2 * p:2 * p + 2, C:C + 128],
                                 kq8[:, 2 * p:2 * p + 2, 0:C],
                                 start=st_, stop=sp_, perf_mode=DR)
                nc.tensor.matmul(ps_s1, kq8[:, 2 * p:2 * p + 2, C + 128:2 * C],
                                 kq8[:, 2 * p:2 * p + 2, 0:C],
                                 start=st_, stop=sp_, perf_mode=DR)

            dump("d_rs1", sc_kq)
            dump("d_kq", kq8[:, 0, :])

            # V channel-major for own half (fp8, carries SW)
            vcm = ap.tile([128, CT, T], F8, tag="vcm")
            for ch in range(4):
                for ct in range(CT):
                    ps = pmm.tile([128, 512], F32, tag="mm")
                    nc.tensor.matmul(ps, wv[:, :, ct * 128:(ct + 1) * 128],
                                     xcm[:, :, ch * 512:(ch + 1) * 512],
                                     start=True, stop=True, perf_mode=DR)
                    with nc.allow_low_precision(reason="fp8 V"):
                        nc.scalar.activation(vcm[:, ct, ch * 512:(ch + 1) * 512],
                                             ps, AF.Identity)

            # S_T evac (bf16) + column sums via Act accumulate
            s_t = stg.tile([128, CT, C], BF16, tag="s_t")
            csum = stg.tile([128, CT], F32, tag="csum")
            with nc.allow_low_precision(reason="bf16 S"):
                nc.scalar.activation(s_t[:, 0, :], ps_s0, AF.Identity,
                                     accum_out=csum[:, 0:1])
                nc.scalar.activation(s_t[:, 1, :], ps_s1, AF.Identity,
                                     accum_out=csum[:, 1:2])
            dump("d_st", s_t)
            dump("d_csum", csum)
            cinv = stg.tile([128, CT], F32, tag="cinv")
            nc.vector.reciprocal(cinv, csum)
            wrs = stg.tile([128, CT, C], BF16, tag="wrs")
            with nc.allow_low_precision(reason="bf16 wrs"):
                nc.gpsimd.tensor_tensor(out=wrs, in0=wr16,
                                        in1=bscale(cinv, 0, CT), op=OP.mult)
            # S2[dk, o] = sum_dq S_T[dq, dk] * wrs[dq, o]   (fp8 out, x SWR*SK8)
            s2 = stg.tile([128, CT, C], F8, tag="s2")
            for mt in range(CT):
                ps = pmm.tile([128, C], F32, tag="mm")
                nc.tensor.matmul(ps, s_t[:, 0, mt * 128:(mt + 1) * 128],
                                 wrs[:, 0, :], start=True, stop=False)
                nc.tensor.matmul(ps, s_t[:, 1, mt * 128:(mt + 1) * 128],
                                 wrs[:, 1, :], start=False, stop=True)
                with nc.allow_low_precision(reason="fp8 S2"):
                    nc.scalar.activation(s2[:, mt, :], ps, AF.Identity)

            # att = V @ S2 ; add1 = x + att * rs1 / (SW * SWR * SK8)
            sc_att = stg.tile([128, NTH], F32, tag="sc_att")
            nc.vector.tensor_scalar(out=sc_att, in0=sc_kq[:, 0:NTH],
                                    scalar1=1.0 / SWR,
                                    scalar2=None, op0=OP.mult)
            add1 = ap.tile([128, NTH, C], F32, tag="residA")
            rs2 = stg.tile([128, NTH], F32, tag="rs_ln2")
            x2n16, x2cm = cm_tiles("m")
            for G in range(4):
                for sg in range(2):
                    g = G * 2 + sg
                    ps = pmm.tile([128, 2, C], F32, tag="mm")
                    for j in range(2):
                        t = g * 2 + j
                        nc.tensor.matmul(ps[:, j, :], vcm[:, :, t * 128:(t + 1) * 128],
                                         s2, start=(j == 0), stop=(j == 1),
                                         perf_mode=DR)
                    for j in range(2):
                        t = g * 2 + j
                        nc.vector.scalar_tensor_tensor(
                            out=add1[:, t, :], in0=ps[:, j, :],
                            scalar=sc_att[:, t:t + 1], in1=xt[:, t, :],
                            op0=OP.mult, op1=OP.add)
                ln_chunk(add1, list(range(NTH)), rs2, G * 4, 4, "ln2")
                cm_group(add1, rs2, x2n16, x2cm, G, "m")

            # ================= MLP 1 =================
            def mlp(resid, xcm16, w_a, w_b, out_tile, final_dma, post_group=None):
                """out = resid + W_b.T @ gelu(W_a.T @ xcm16) / SW.
                post_group(G) is emitted after each 4-tile out group."""
                h8 = ap.tile([128, FT, T], F8, tag="h8")
                for ch in range(4):
                    for ft in range(FT):
                        ps = pmm.tile([128, 512], F32, tag="mm")
                        for kt in range(CT):
                            nc.tensor.matmul(
                                ps, w_a[:, kt, ft * 128:(ft + 1) * 128],
                                xcm16[:, kt, ch * 512:(ch + 1) * 512],
                                start=(kt == 0), stop=(kt == CT - 1))
                        with nc.allow_low_precision(reason="fp8 h"):
                            nc.scalar.activation(h8[:, ft, ch * 512:(ch + 1) * 512],
                                                 ps, AF.Gelu, scale=1.0 / SW)
                for G in range(4):
                    for sg in range(2):
                        g = G * 2 + sg
                        ps = pmm.tile([128, 2, C], F32, tag="mm")
                        for j in range(2):
                            t = g * 2 + j
                            for fp in range(FT // 2):
                                nc.tensor.matmul(
                                    ps[:, j, :],
                                    h8[:, 2 * fp:2 * fp + 2, t * 128:(t + 1) * 128],
                                    w_b[:, 2 * fp:2 * fp + 2, :],
                                    start=(fp == 0 and j == 0),
                                    stop=(fp == FT // 2 - 1 and j == 1),
                                    perf_mode=DR)
                        if final_dma:
                            ot = tp.tile([128, 2, C], F32, tag="out_sb", bufs=4)
                            nc.vector.scalar_tensor_tensor(
                                out=ot, in0=ps, scalar=1.0 / SW,
                                in1=resid[:, g * 2:g * 2 + 2, :],
                                op0=OP.mult, op1=OP.add)
                            nc.sync.dma_start(
                                out=io["y"][:, :].rearrange(
                                    "(tt p) c -> p tt c", p=128)[:, g * 2:g * 2 + 2, :],
                                in_=ot)
                        else:
                            nc.vector.scalar_tensor_tensor(
                                out=out_tile[:, g * 2:g * 2 + 2, :], in0=ps,
                                scalar=1.0 / SW, in1=resid[:, g * 2:g * 2 + 2, :],
                                op0=OP.mult, op1=OP.add)
                    if post_group is not None:
                        post_group(G)
                return out_tile

            rs2 = ln_rs(add1, list(range(NTH)), "ln2", chunk=4)
            dump("d_rs2", rs2)
            add2 = ap.tile([128, NTH, C], F32, tag="residB")
            rs3 = stg.tile([128, NTH], F32, tag="rs_ln3")
            x3n16, n3cm = cm_tiles("ca")

            def post_mlp1(G):
                ln_chunk(add2, list(range(NTH)), rs3, G * 4, 4, "ln3")
                cm_group(add2, rs3, x3n16, n3cm, G, "ca")

            mlp(add1, x2cm, w1, w2, add2, False, post_group=post_mlp1)

            # ================= ChannelAttention =================
            dump("d_add2", add2[:, 0, :])
            qkw = wload("qk8", CT, 2 * C)
            vvw = wload("vv8", CT, C)
            pjw = wload("pj8", CT, C)
            bdm = wp.tile([128, CT, 128], BF16, tag="bdm")
            nc.sync.dma_start(out=bdm, in_=io["bdmask"][:, :, :])
            temp_sb = wp.tile([128, CT], F32, tag="temp")
            nc.sync.dma_start(out=temp_sb, in_=io["temp_c"][:, :])


            # qk projections (fp8 x SQK); gram + norms accumulated on PE
            qkt8 = ap.tile([128, NTH, 2 * C], F8, tag="qkt8")
            ps_a0 = pacc.tile([128, C], F32, tag="ps_s0")
            ps_a1 = pacc.tile([128, C], F32, tag="ps_s1")
            ps_nrm = pacc.tile([1, 2 * C], F32, tag="ps_nrm")
            sq16 = ap.tile([128, NTH, 2 * C], BF16, tag="sq16")
            for p in range(NTH // 2):
                for i in range(2):
                    t = 2 * p + i
                    ps = pmm.tile([128, 2 * C], F32, tag="mm")
                    for kt in range(CT):
                        nc.tensor.matmul(ps, n3cm[:, kt, t * 128:(t + 1) * 128],
                                         qkw[:, kt, :], start=(kt == 0),
                                         stop=(kt == CT - 1))
                    with nc.allow_low_precision(reason="fp8 qk"):
                        nc.scalar.activation(qkt8[:, t, :], ps, AF.Identity)
                        nc.scalar.activation(sq16[:, t, :], ps, AF.Square)
                st_, sp_ = (p == 0), (p == NTH // 2 - 1)
                nc.tensor.matmul(ps_nrm, ones8, sq16[:, 2 * p, :],
                                 start=st_, stop=False)
                nc.tensor.matmul(ps_nrm, ones8, sq16[:, 2 * p + 1, :],
                                 start=False, stop=sp_)
                nc.tensor.matmul(ps_a0, qkt8[:, 2 * p:2 * p + 2, 0:128],
                                 qkt8[:, 2 * p:2 * p + 2, C:2 * C],
                                 start=st_, stop=sp_, perf_mode=DR)
                nc.tensor.matmul(ps_a1, qkt8[:, 2 * p:2 * p + 2, 128:C],
                                 qkt8[:, 2 * p:2 * p + 2, C:2 * C],
                                 start=st_, stop=sp_, perf_mode=DR)

            dump("d_qkt", qkt8[:, 0, :])

            # pack the used per-head diagonal 32x32 gram blocks + norms
            ca_tx = stg.tile([128, CCW], F32, tag="ca_tx")
            for hh in range(H_CH):
                ct, r0 = hh // 4, (hh % 4) * HD
                src_ps = ps_a0 if ct == 0 else ps_a1
                nc.vector.tensor_copy(ca_tx[r0:r0 + HD, ct * HD:(ct + 1) * HD],
                                      src_ps[r0:r0 + HD, hh * HD:(hh + 1) * HD])
            nrm_sb = stg.tile([1, 2 * C], F32, tag="nrm_sb")
            nc.vector.tensor_copy(nrm_sb, ps_nrm)
            ps_fl = pmm.tile([128, 2 * CT], F32, tag="mm", name="ps_fl")
            for i in range(2 * CT):
                nc.tensor.matmul(ps_fl[:, i:i + 1],
                                 nrm_sb[0:1, i * 128:(i + 1) * 128],
                                 ident32[0:1, 0:1], is_transpose=True,
                                 start=(i == 0), stop=(i == 2 * CT - 1))
            nc.vector.tensor_copy(ca_tx[:, 2 * HD:CCW], ps_fl)
            nc.sync.dma_start(out=cc_in[:, :], in_=ca_tx[:, :])
            nc.gpsimd.collective_compute(
                "AllGather", OP.bypass, replica_groups=REPLICA_GROUPS,
                ins=[cc_in[:, :]], outs=[cc_out[:, :, :]])

            # MLP2 weights arrive during the collective
            w3 = wload("w3_8", CT, DFF, tag="wmlp_a")
            w4 = wload("w4_8", FT, C, tag="wmlp_b")

            # v channel-major (overlaps the collective)
            vcm3 = ap.tile([128, CT, T], F8, tag="vcm")
            for ch in range(4):
                for ct in range(CT):
                    ps = pmm.tile([128, 512], F32, tag="mm")
                    for kt in range(CT):
                        nc.tensor.matmul(ps, vvw[:, kt, ct * 128:(ct + 1) * 128],
                                         n3cm[:, kt, ch * 512:(ch + 1) * 512],
                                         start=(kt == 0), stop=(kt == CT - 1))
                    with nc.allow_low_precision(reason="fp8 v"):
                        nc.scalar.activation(vcm3[:, ct, ch * 512:(ch + 1) * 512],
                                             ps, AF.Identity)


            # ---- post-collective epilogue ----
            ca_rx = stg.tile([128, 2, CCW], F32, tag="ca_rx")
            nc.sync.dma_start(out=ca_rx, in_=cc_out[:, :, :].rearrange("r p w -> p r w"))
            ca_tot = stg.tile([128, CCW], F32, tag="ca_tot")
            nc.vector.tensor_tensor(out=ca_tot, in0=ca_rx[:, 0, :],
                                    in1=ca_rx[:, 1, :], op=OP.add)

            dump("d_catot", ca_tot)
            nktot = ca_tot[:, 2 * HD:CCW]    # [128, 4]: qsumsq-cols | ksumsq-cols
            # inv norms via Newton rsqrt (columns, f32)
            invn = stg.tile([128, 2 * CT], F32, tag="invn")
            nw_t = tp.tile([128, 2 * CT], F32, tag="nw_t")
            nc.vector.tensor_scalar(out=invn.bitcast(I32), in0=nktot.bitcast(I32),
                                    scalar1=1, scalar2=None,
                                    op0=OP.logical_shift_right)
            nc.vector.tensor_scalar(out=invn.bitcast(I32), in0=invn.bitcast(I32),
                                    scalar1=-1, scalar2=MAGIC,
                                    op0=OP.mult, op1=OP.add)
            for _ in range(2):
                nc.vector.tensor_tensor(out=nw_t, in0=invn, in1=invn, op=OP.mult)
                nc.vector.tensor_tensor(out=nw_t, in0=nw_t, in1=nktot, op=OP.mult)
                nc.vector.tensor_scalar(out=nw_t, in0=nw_t, scalar1=-0.5,
                                        scalar2=1.5, op0=OP.mult, op1=OP.add)
                nc.vector.tensor_tensor(out=invn, in0=invn, in1=nw_t, op=OP.mult)
            # scale invq by temperature and the poly-exp 1/4 folding
            invq = stg.tile([128, CT], F32, tag="invq")
            nc.vector.tensor_tensor(out=invq, in0=invn[:, 0:CT], in1=temp_sb,
                                    op=OP.mult)
            nc.vector.tensor_scalar(out=invq, in0=invq, scalar1=0.25,
                                    scalar2=None, op0=OP.mult)
            # invk back to a row [1, C] via PE transpose, broadcast to [128, C]
            ps_kf = pmm.tile([1, C], F32, tag="mm", name="ps_kf")
            for ct in range(CT):
                nc.tensor.matmul(ps_kf[0:1, ct * 128:(ct + 1) * 128],
                                 invn[:, CT + ct:CT + ct + 1],
                                 ident32, is_transpose=True,
                                 start=(ct == 0), stop=(ct == 1))
            invk_row = tp.tile([1, C], F32, tag="invk_row")
            nc.vector.tensor_copy(invk_row, ps_kf)
            ones_row16 = tp.tile([1, 128], BF16, tag="ones_row16")
            nc.vector.memset(ones_row16, 1.0)
            invk_row16 = tp.tile([1, C], BF16, tag="invk_row16")
            with nc.allow_low_precision(reason="bf16 bcast operand"):
                nc.vector.tensor_copy(invk_row16, invk_row)
            ps_bk = pmm.tile([128, C], F32, tag="mm", name="ps_bk")
            nc.tensor.matmul(ps_bk, ones_row16, invk_row16, start=True, stop=True)
            bk = stg.tile([128, C], F32, tag="bk")
            nc.vector.tensor_copy(bk, ps_bk)

            # logits = gram * invq(part) * invk(elem); per-head blocks only
            attn_l = stg.tile([128, CT, 128], F32, tag="attn_l")
            nc.vector.memset(attn_l, 0.0)
            for hh in range(H_CH):
                ct, r0 = hh // 4, (hh % 4) * HD
                nc.vector.scalar_tensor_tensor(
                    out=attn_l[r0:r0 + HD, ct, r0:r0 + HD],
                    in0=ca_tot[r0:r0 + HD, ct * HD:(ct + 1) * HD],
                    scalar=invq[r0:r0 + HD, ct:ct + 1],
                    in1=bk[r0:r0 + HD, hh * HD:(hh + 1) * HD],
                    op0=OP.mult, op1=OP.mult)

            # exp via (1 + u + u^2/2 + u^3/6)^4; u = logits/4 folded into invq
            u = attn_l
            pe_ = stg.tile([128, CT, 128], F32, tag="attn_p")
            nc.vector.tensor_scalar(out=pe_, in0=u, scalar1=1.0 / 6.0,
                                    scalar2=0.5, op0=OP.mult, op1=OP.add)
            nc.vector.tensor_tensor(out=pe_, in0=pe_, in1=u, op=OP.mult)
            nc.vector.tensor_scalar(out=pe_, in0=pe_, scalar1=1.0, scalar2=None,
                                    op0=OP.add)
            nc.vector.tensor_tensor(out=pe_, in0=pe_, in1=u, op=OP.mult)
            nc.vector.tensor_scalar(out=pe_, in0=pe_, scalar1=1.0, scalar2=None,
                                    op0=OP.add)
            nc.vector.tensor_tensor(out=pe_, in0=pe_, in1=pe_, op=OP.mult)
            nc.vector.tensor_tensor(out=pe_, in0=pe_, in1=pe_, op=OP.mult)
            # mask off-block entries, row-normalize
            with nc.allow_low_precision(reason="bf16 mask"):
                nc.vector.tensor_tensor(out=pe_, in0=pe_, in1=bdm, op=OP.mult)
            rsum = tp.tile([128, CT], F32, tag="attn_rs")
            nc.vector.tensor_reduce(rsum.unsqueeze(-1), pe_, axis=AX.X, op=OP.add)
            rinv = tp.tile([128, CT], F32, tag="attn_ri")
            nc.vector.reciprocal(rinv, rsum)
            attn16 = stg.tile([128, CT, 128], BF16, tag="attn16")
            with nc.allow_low_precision(reason="bf16 attn"):
                nc.vector.tensor_tensor(out=attn16, in0=pe_,
                                        in1=bscale(rinv, 0, CT, 128), op=OP.mult)

            dump("d_attn", attn16)

            # transpose attn blocks -> lhsT; attn @ v ; proj ; add3
            at_bd = stg.tile([128, CT, 128], BF16, tag="at_bd")
            for ct in range(CT):
                ps_at = pmmt.tile([128, 128], BF16, tag="mmt", name=f"ps_at{ct}")
                nc.tensor.transpose(ps_at, attn16[:, ct, :], ident16)
                with nc.allow_low_precision(reason="bf16 attn lhsT"):
                    nc.vector.tensor_copy(at_bd[:, ct, :], ps_at)

            cac = ap.tile([128, CT, T], F8, tag="cac")
            for ct in range(CT):
                for ch in range(4):
                    ps = pmm.tile([128, 512], F32, tag="mm")
                    nc.tensor.matmul(ps, at_bd[:, ct, :],
                                     vcm3[:, ct, ch * 512:(ch + 1) * 512],
                                     start=True, stop=True)
                    with nc.allow_low_precision(reason="fp8 cac"):
                        nc.scalar.activation(cac[:, ct, ch * 512:(ch + 1) * 512],
                                             ps, AF.Identity)

            add3 = ap.tile([128, NTH, C], F32, tag="residA")
            rs4 = stg.tile([128, NTH], F32, tag="rs_ln4")
            x4n16, x4cm = cm_tiles("f")
            for G in range(4):
                for sg in range(2):
                    g = G * 2 + sg
                    ps = pmm.tile([128, 2, C], F32, tag="mm")
                    for j in range(2):
                        t = g * 2 + j
                        nc.tensor.matmul(ps[:, j, :], cac[:, :, t * 128:(t + 1) * 128],
                                         pjw, start=(j == 0), stop=(j == 1),
                                         perf_mode=DR)
                    nc.vector.scalar_tensor_tensor(
                        out=add3[:, g * 2:g * 2 + 2, :], in0=ps,
                        scalar=1.0 / (SW * SW), in1=add2[:, g * 2:g * 2 + 2, :],
                        op0=OP.mult, op1=OP.add)
                ln_chunk(add3, list(range(NTH)), rs4, G * 4, 4, "ln4")
                cm_group(add3, rs4, x4n16, x4cm, G, "f")

            # ================= MLP 2 (writes y) =================
            mlp(add3, x4cm, w3, w4, None, True)

    nc.compile()
    _CACHE["nc"] = nc
    return nc


def prep_host(inputs):
    """Weight/layout prep shared by all cores (no arithmetic on x)."""
    f8 = ml_dtypes.float8_e4m3
    f = lambda k: np.asarray(inputs[k], np.float32)
    for k in ("ln1_b", "ln2_b", "ln3_b", "ln4_b", "m1_b1", "m1_b2",
              "m2_b1", "m2_b2", "proj_b"):
        assert np.abs(f(k)).max() == 0.0, f"{k} nonzero; bias path not emitted"
    g1, g2, g3, g4 = f("ln1_g"), f("ln2_g"), f("ln3_g"), f("ln4_g")

    def fold(w, g, center=True):
        """M = Pc @ (diag(g) @ w.T): [in, out] with LN gamma + centering."""
        m = (w * g[None, :]).T.astype(np.float32)
        if center:
            m = m - m.mean(axis=0, keepdims=True)
        return m

    def arr_ct(m, scale):
        """[C_in, O] -> [128, C_in//128, O] fp8 with scale."""
        ci, o = m.shape
        return np.ascontiguousarray(
            (m * scale).reshape(ci // 128, 128, o).transpose(1, 0, 2)
        ).astype(f8)

    qkv_w = f("qkv_w")
    wkq = np.concatenate([fold(f("wk"), g1), fold(f("wq"), g1)], axis=1)
    qk_ca = fold(qkv_w[:2 * C], g3)          # [C, 2C]
    vv_ca = fold(qkv_w[2 * C:], g3)          # [C, C]
    w1 = fold(f("m1_w1"), g2)
    w3 = fold(f("m2_w1"), g4)
    w2 = f("m1_w2").T                        # [DFF, C]
    w4 = f("m2_w2").T
    wr = f("wr").T                           # [C, C]
    pj = f("proj_w").T

    bdmask = np.zeros((128, CT, 128), np.float32)
    for hh in range(H_CH):
        ct, r0 = hh // 4, (hh % 4) * HD
        bdmask[r0:r0 + HD, ct, r0:r0 + HD] = 1.0

    temp = np.repeat(f("temperature").reshape(H_CH), HD).reshape(CT, 128).T

    return {
        "wkq8": arr_ct(wkq, SW),
        "wv8": arr_ct(fold(f("wv"), g1), SW),
        "wr16": np.ascontiguousarray(
            (wr * SWR).reshape(CT, 128, C).transpose(1, 0, 2)
        ).astype(ml_dtypes.bfloat16),
        "qk8": arr_ct(qk_ca, SQK),
        "vv8": arr_ct(vv_ca, SW),
        "pj8": arr_ct(pj, SW),
        "w1_8": arr_ct(w1, SW),
        "w2_8": arr_ct(w2, SW),
        "w3_8": arr_ct(w3, SW),
        "w4_8": arr_ct(w4, SW),
        "ident8": np.eye(128, dtype=f8),
        "ident16": np.eye(128, dtype=ml_dtypes.bfloat16),
        "ones8": np.ones((128, 1), f8),
        "bdmask": bdmask.astype(ml_dtypes.bfloat16),
        "temp_c": np.ascontiguousarray(temp, np.float32),
        "ident32": np.eye(128, dtype=np.float32),
    }


def make_in_maps(inputs):
    shared = prep_host(inputs)
    f8 = ml_dtypes.float8_e4m3
    x = np.asarray(inputs["x"], np.float32)
    in_maps = []
    for c in range(NCORES):
        b, hlf = c // 2, c % 2
        # own half first, peer half second (layout only, no arithmetic)
        xo = np.concatenate([x[b, hlf * T:(hlf + 1) * T, :],
                             x[b, (1 - hlf) * T:(2 - hlf) * T, :]], axis=0)
        m = dict(shared)
        m["xt16"] = np.ascontiguousarray(
            xo.reshape(NT, 128, C).transpose(1, 0, 2)).astype(ml_dtypes.bfloat16)
        m["xcm8"] = np.ascontiguousarray(
            xo.T.reshape(CT, 128, N).transpose(1, 0, 2)).astype(f8)
        in_maps.append(m)
    return in_maps


def assemble(results):
    y = np.empty((B, N, C), np.float32)
    for c in range(NCORES):
        b, hlf = c // 2, c % 2
        y[b, hlf * T:(hlf + 1) * T, :] = results[c]["y"]
    return y


def kernel(**inputs):
    from concourse.bass_utils import run_bass_kernel_spmd

    nc = build_program()
    in_maps = make_in_maps(inputs)
    res = run_bass_kernel_spmd(nc, in_maps, list(range(NCORES)))
    return assemble(res.results)


# revision 23
# speedup vs baseline: 1.0237x; 1.0237x over previous
"""DualTransformerBlock Trainium2 kernel (v2).

Strategy (8 cores: core c -> sample b=c//2, token half h=c%2, T=2048):
  - EfficientAttention reassociated: att = V @ ((K^T Q)/colsum @ wr.T); the
    [N,N] context never materializes.  The K/Q-side stats (exp projections and
    the [C,C] S matrix) are computed REDUNDANTLY for all 4096 tokens on both
    cores of a pair, so no collective is needed for EA.
  - Each core's x tensors are ordered with ITS half first (tiles 0..15 = own
    half, 16..31 = peer half); the S/stat sums are order-invariant, so one
    SPMD program serves all cores.
  - ChannelAttention still needs one cross-half reduction (per-head gram +
    q/k norms); exchanged via AllGather (cheaper than AllReduce in the cost
    model) and summed locally.
  - LayerNorm: mean-centering is folded into the following weight matrices on
    the host ((x-mean) @ W == x @ (Pc W)); the per-token 1/sigma comes from a
    bit-trick Newton rsqrt (no activation-table use) and is fused into Act
    scale= or evacuation scalars.
  - Heavy matmuls run in fp8e4m3 with DoubleRow perf mode (2 contraction
    tiles per instruction, 0.5 cyc/row).  Host-side exponent scaling keeps
    fp8 operands out of the subnormal range; scales cancel or fold into
    per-token evacuation scalars.
  - Activation tables: only Exp (EA) and Gelu (MLPs) -> 2 table loads.  The
    tiny per-head channel-attn softmax uses a polynomial exp on DVE/Pool.
"""

import os
import sys

sys.path.insert(0, "/opt/trn_rl_repo")

import numpy as np
import ml_dtypes

import concourse.bass as bass
import concourse.mybir as mybir
from concourse import bacc
from concourse.tile import TileContext

F32 = mybir.dt.float32
BF16 = mybir.dt.bfloat16
F8 = mybir.dt.float8e4
I32 = mybir.dt.int32
AF = mybir.ActivationFunctionType
OP = mybir.AluOpType
AX = mybir.AxisListType
DR = mybir.MatmulPerfMode.DoubleRow

B, N, C = 4, 4096, 256
H_CH = 8
HD = C // H_CH          # 32
DFF = 4 * C             # 1024
EPS_LN = 1e-5

NCORES = 8
T = N // 2              # 2048 tokens per core half
NT = N // 128           # 32 token tiles (full sample)
NTH = T // 128          # 16 token tiles (own half)
CT = C // 128           # 2 channel tiles
FT = DFF // 128         # 8 ff tiles
REPLICA_GROUPS = [[0, 1], [2, 3], [4, 5], [6, 7]]
CCW = 2 * HD + 2 * CT   # collective payload width (68)

# fp8 exponent scales (host folds these into weights; device descales)
SW = 32.0       # generic weight scale (wkq, wv, w1..w4, qkv-v, proj)
SQK = 8.0       # channel-attn qk scale
SK8 = 64.0      # normalized-k softmax scale
SWR = 256.0     # wr scale
MAGIC = 0x5F3759DF

_CACHE = {}


def build_program():
    if "nc" in _CACHE:
        return _CACHE["nc"]
    nc = bacc.Bacc(None, target_bir_lowering=False)

    io = {}

    def param(name, shape, dt=F32):
        io[name] = nc.declare_dram_parameter(name, list(shape), dt, isOutput=False)

    param("xt16", (128, NT, C), BF16)       # token-major x, all tokens, own half first
    param("xcm8", (128, CT, N), F8)         # channel-major x, all tokens
    param("wkq8", (128, CT, 2 * C), F8)     # [K|Q] proj  (centered, g1, xSW)
    param("wv8", (128, CT, C), F8)          # EA values proj (centered, g1, xSW)
    param("wr16", (128, CT, C), BF16)       # EA out proj (xSWR)
    param("qk8", (128, CT, 2 * C), F8)      # CA [q|k] proj (centered, g3, xSQK)
    param("vv8", (128, CT, C), F8)          # CA v proj (centered, g3, xSW)
    param("pj8", (128, CT, C), F8)          # CA out proj (xSW)
    param("w1_8", (128, CT, DFF), F8)       # MLP1 fc1 (centered, g2, xSW)
    param("w2_8", (128, FT, C), F8)         # MLP1 fc2 (xSW)
    param("w3_8", (128, CT, DFF), F8)       # MLP2 fc1 (centered, g4, xSW)
    param("w4_8", (128, FT, C), F8)         # MLP2 fc2 (xSW)
    param("ident8", (128, 128), F8)
    param("ident16", (128, 128), BF16)
    param("ident32", (128, 128), F32)
    param("ones8", (128, 1), F8)
    param("bdmask", (128, CT, 128), BF16)   # per-head 32x32 block-diag mask
    param("temp_c", (128, CT), F32)         # CA temperature, column layout
    io["y"] = nc.declare_dram_parameter("y", [T, C], F32, isOutput=True)
    DBG = os.environ.get("KDBG", "0") == "1"
    dbg_specs = {
        "d_rs1": (128, NT), "d_kq": (128, 2 * C), "d_st": (128, CT, C),
        "d_csum": (128, CT), "d_s2": (128, CT, C), "d_add1": (128, C),
        "d_vcm": (128, CT, 512), "d_add2": (128, C), "d_qkt": (128, 2 * C),
        "d_catot": (128, CCW), "d_attn": (128, CT, 128), "d_add3": (128, C),
        "d_rs2": (128, NTH),
    }
    if DBG:
        for k, shp in dbg_specs.items():
            io[k] = nc.declare_dram_parameter(k, list(shp), F32, isOutput=True)

    cc_in = nc.dram_tensor("cc_in", [128, CCW], F32)
    cc_out = nc.dram_tensor("cc_out", [2, 128, CCW], F32)

    with TileContext(nc) as tc:
        with (
            tc.tile_pool(name="wpool", bufs=1) as wp,
            tc.tile_pool(name="apool", bufs=1) as ap,
            tc.tile_pool(name="tmp", bufs=3) as tp,
            tc.tile_pool(name="stage", bufs=1) as stg,
            tc.tile_pool(name="pacc", bufs=1, space="PSUM") as pacc,
            tc.tile_pool(name="pmm", bufs=3, space="PSUM") as pmm,
            tc.tile_pool(name="pmmt", bufs=2, space="PSUM") as pmmt,
        ):
            # ---------------- input DMA ----------------
            xt = ap.tile([128, NT, C], BF16, tag="xt")
            for g in range(4):
                nc.sync.dma_start(out=xt[:, g * 8:(g + 1) * 8, :],
                                  in_=io["xt16"][:, g * 8:(g + 1) * 8, :])
            xcm = ap.tile([128, CT, N], F8, tag="xcm")
            for g in range(4):
                nc.sync.dma_start(
                    out=xcm[:, :, g * (N // 4):(g + 1) * (N // 4)],
                    in_=io["xcm8"][:, :, g * (N // 4):(g + 1) * (N // 4)])

            def wload(name, d1, d2, dt=F8, tag=None):
                t = wp.tile([128, d1, d2], dt, tag=tag or name)
                nc.sync.dma_start(out=t, in_=io[name][:, :, :])
                return t

            wkq = wload("wkq8", CT, 2 * C)
            wv = wload("wv8", CT, C)
            wr16 = wload("wr16", CT, C, BF16)
            w1 = wload("w1_8", CT, DFF, tag="wmlp_a")
            w2 = wload("w2_8", FT, C, tag="wmlp_b")
            ident8 = wp.tile([128, 128], F8, tag="ident8")
            nc.sync.dma_start(out=ident8, in_=io["ident8"][:, :])
            ident16 = wp.tile([128, 128], BF16, tag="ident16")
            nc.sync.dma_start(out=ident16, in_=io["ident16"][:, :])
            ident32 = wp.tile([128, 128], F32, tag="ident32")
            nc.sync.dma_start(out=ident32, in_=io["ident32"][:, :])
            ones8 = wp.tile([128, 1], F8, tag="ones8")
            nc.sync.dma_start(out=ones8, in_=io["ones8"][:, :])

            def dump(name, src_ap):
                if DBG:
                    dt = stg.tile(list(src_ap.shape), F32, tag=f"dbg_{name}",
                                  name=f"dbg_{name}", bufs=1)
                    nc.vector.tensor_copy(dt, src_ap)
                    nc.sync.dma_start(out=io[name].__getitem__(
                        tuple([slice(None)] * len(src_ap.shape))), in_=dt)

            # ---------------- helpers ----------------
            def ln_chunk(src_t, tiles, y, c0, m, tag, div=1.0):
                """Write rsqrt(var+eps)/div for tiles[c0:c0+m] into y[:, c0:c0+m]."""
                mvg = tp.tile([128, m, 2], F32, tag=f"mvg_{tag}",
                              name=f"mvg_{tag}_{c0}", bufs=2)
                for j in range(m):
                    st6 = tp.tile([128, 6], BF16, tag="st6", bufs=4)
                    nc.vector.bn_stats(out=st6, in_=src_t[:, tiles[c0 + j], :])
                    nc.vector.bn_aggr(out=mvg[:, j, :], in_=st6)
                vpe = tp.tile([128, m], F32, tag=f"vpe_{tag}",
                              name=f"vpe_{tag}_{c0}", bufs=2)
                nc.vector.tensor_scalar(out=vpe, in0=mvg[:, :, 1],
                                        scalar1=EPS_LN, scalar2=None,
                                        op0=OP.add)
                ys = y[:, c0:c0 + m]
                nc.vector.tensor_scalar(out=ys.bitcast(I32),
                                        in0=vpe.bitcast(I32),
                                        scalar1=1, scalar2=None,
                                        op0=OP.logical_shift_right)
                nc.vector.tensor_scalar(out=ys.bitcast(I32),
                                        in0=ys.bitcast(I32),
                                        scalar1=-1, scalar2=MAGIC,
                                        op0=OP.mult, op1=OP.add)
                t_ = tp.tile([128, m], F32, tag=f"nt_{tag}",
                             name=f"nt_{tag}_{c0}", bufs=2)
                for it in range(2):
                    last = it == 1
                    nc.gpsimd.tensor_tensor(out=t_, in0=ys, in1=ys, op=OP.mult)
                    nc.gpsimd.tensor_tensor(out=t_, in0=t_, in1=vpe, op=OP.mult)
                    nc.vector.tensor_scalar(
                        out=t_, in0=t_,
                        scalar1=(-0.5 / div) if last else -0.5,
                        scalar2=(1.5 / div) if last else 1.5,
                        op0=OP.mult, op1=OP.add)
                    nc.gpsimd.tensor_tensor(out=ys, in0=ys, in1=t_, op=OP.mult)

            def ln_rs(src_t, tiles, tag, div=1.0, chunk=8):
                n = len(tiles)
                y = stg.tile([128, n], F32, tag=f"rs_{tag}", name=f"rs_{tag}")
                for c0 in range(0, n, chunk):
                    ln_chunk(src_t, tiles, y, c0, min(chunk, n - c0), tag, div)
                return y

            def bscale(sc, j0, n, width=C):
                """[128, n] slice of sc -> stride-0 broadcast [128, n, width]."""
                return sc[:, j0:j0 + n].unsqueeze(-1).broadcast_to((128, n, width))

            def cm_group(src_t, rs, x16, cm, g, tag):
                """tiles 4g..4g+4: mult (alt Pool/DVE) + DMA xbar transposes."""
                with nc.allow_low_precision(reason="bf16 matmul operand"):
                    eng = nc.gpsimd if g % 2 == 0 else nc.vector
                    eng.tensor_tensor(
                        out=x16[:, g * 4:(g + 1) * 4, :],
                        in0=src_t[:, g * 4:(g + 1) * 4, :],
                        in1=bscale(rs, g * 4, 4), op=OP.mult)
                    for t in range(g * 4, g * 4 + 4):
                        nc.sync.dma_start_transpose(
                            cm[:, :, t * 128:(t + 1) * 128], x16[:, t, :])

            def cm_tiles(tag):
                x16 = ap.tile([128, NTH, C], BF16, tag=f"tm_{tag}",
                              name=f"tm_{tag}")
                cm = ap.tile([128, CT, T], BF16, tag=f"cm_{tag}",
                             name=f"cm_{tag}")
                return x16, cm

            # ================= EfficientAttention =================
            # ================= EfficientAttention =================
            # ================= EfficientAttention =================
            # sc_kq = rs1/SW for ALL 32 tiles (stats replicated across pair);
            # LN1 chunks are interleaved into the projection loop below.
            sc_kq = stg.tile([128, NT], F32, tag="rs_ln1")

            kq8 = ap.tile([128, NT, 2 * C], F8, tag="kq8")
            ksums = stg.tile([128, NT], F32, tag="ksums")
            rinv64 = stg.tile([128, NT], F32, tag="rinv64")
            ps_s0 = pacc.tile([128, C], F32, tag="ps_s0")
            ps_s1 = pacc.tile([128, C], F32, tag="ps_s1")

            for p in range(NT // 2):
                if p % 4 == 0:
                    ln_chunk(xt, list(range(NT)), sc_kq, p * 2, 8, "ln1", div=SW)
                for i in range(2):
                    t = 2 * p + i
                    ps = pmm.tile([128, 2 * C], F32, tag="mm")
                    nc.tensor.matmul(ps, xcm[:, :, t * 128:(t + 1) * 128],
                                     wkq, start=True, stop=True, perf_mode=DR)
                    with nc.allow_low_precision(reason="fp8 exp"):
                        nc.scalar.activation(kq8[:, t, :], ps, AF.Exp,
                                             scale=sc_kq[:, t:t + 1])
                # ksum + SK8/ksum for the pair
                nc.vector.tensor_reduce(
                    ksums[:, 2 * p:2 * p + 2].unsqueeze(-1),
                    kq8[:, 2 * p:2 * p + 2, 0:C], axis=AX.X, op=OP.add)
                nc.vector.reciprocal(rinv64[:, 2 * p:2 * p + 2],
                                     ksums[:, 2 * p:2 * p + 2])
                nc.vector.tensor_scalar(out=rinv64[:, 2 * p:2 * p + 2],
                                        in0=rinv64[:, 2 * p:2 * p + 2],
                                        scalar1=SK8, scalar2=None, op0=OP.mult)
                with nc.allow_low_precision(reason="fp8 softmax-k scale"):
                    keng = nc.vector if p % 3 == 0 else nc.gpsimd
                    keng.tensor_tensor(
                        out=kq8[:, 2 * p:2 * p + 2, 0:C],
                        in0=kq8[:, 2 * p:2 * p + 2, 0:C],
                        in1=bscale(rinv64, 2 * p, 2), op=OP.mult)
                # S accumulation (DoubleRow over the token-tile pair)
                st_, sp_ = (p == 0), (p == NT // 2 - 1)
                nc.tensor.matmul(ps_s0, kq8[:, 2 * p:2 * p + 2, C:C + 128],
                                 kq8[:, 2 * p:2 * p + 2, 0:C],
                                 start=st_, stop=sp_, perf_mode=DR)
                nc.tensor.matmul(ps_s1, kq8[:, 2 * p:2 * p + 2, C + 128:2 * C],
                                 kq8[:, 2 * p:2 * p + 2, 0:C],
                                 start=st_, stop=sp_, perf_mode=DR)

            dump("d_rs1", sc_kq)
            dump("d_kq", kq8[:, 0, :])

            # V channel-major for own half (fp8, carries SW)
            vcm = ap.tile([128, CT, T], F8, tag="vcm")
            for ch in range(4):
                for ct in range(CT):
                    ps = pmm.tile([128, 512], F32, tag="mm")
                    nc.tensor.matmul(ps, wv[:, :, ct * 128:(ct + 1) * 128],
                                     xcm[:, :, ch * 512:(ch + 1) * 512],
                                     start=True, stop=True, perf_mode=DR)
                    with nc.allow_low_precision(reason="fp8 V"):
                        nc.scalar.activation(vcm[:, ct, ch * 512:(ch + 1) * 512],
                                             ps, AF.Identity)

            # S_T evac (bf16) + column sums via Act accumulate
            s_t = stg.tile([128, CT, C], BF16, tag="s_t")
            csum = stg.tile([128, CT], F32, tag="csum")
            with nc.allow_low_precision(reason="bf16 S"):
                nc.scalar.activation(s_t[:, 0, :], ps_s0, AF.Identity,
                                     accum_out=csum[:, 0:1])
                nc.scalar.activation(s_t[:, 1, :], ps_s1, AF.Identity,
                                     accum_out=csum[:, 1:2])
            dump("d_st", s_t)
            dump("d_csum", csum)
            cinv = stg.tile([128, CT], F32, tag="cinv")
            nc.vector.reciprocal(cinv, csum)
            wrs = stg.tile([128, CT, C], BF16, tag="wrs")
            with nc.allow_low_precision(reason="bf16 wrs"):
                nc.gpsimd.tensor_tensor(out=wrs, in0=wr16,
                                        in1=bscale(cinv, 0, CT), op=OP.mult)
            # S2[dk, o] = sum_dq S_T[dq, dk] * wrs[dq, o]   (fp8 out, x SWR*SK8)
            s2 = stg.tile([128, CT, C], F8, tag="s2")
            for mt in range(CT):
                ps = pmm.tile([128, C], F32, tag="mm")
                nc.tensor.matmul(ps, s_t[:, 0, mt * 128:(mt + 1) * 128],
                                 wrs[:, 0, :], start=True, stop=False)
                nc.tensor.matmul(ps, s_t[:, 1, mt * 128:(mt + 1) * 128],
                                 wrs[:, 1, :], start=False, stop=True)
                with nc.allow_low_precision(reason="fp8 S2"):
                    nc.scalar.activation(s2[:, mt, :], ps, AF.Identity)

            # att = V @ S2 ; add1 = x + att * rs1 / (SW * SWR * SK8)
            sc_att = stg.tile([128, NTH], F32, tag="sc_att")
            nc.vector.tensor_scalar(out=sc_att, in0=sc_kq[:, 0:NTH],
                                    scalar1=1.0 / SWR,
                                    scalar2=None, op0=OP.mult)
            add1 = ap.tile([128, NTH, C], F32, tag="residA")
            rs2 = stg.tile([128, NTH], F32, tag="rs_ln2")
            x2n16, x2cm = cm_tiles("m")
            for G in range(4):
                for sg in range(2):
                    g = G * 2 + sg
                    ps = pmm.tile([128, 2, C], F32, tag="mm")
                    for j in range(2):
                        t = g * 2 + j
                        nc.tensor.matmul(ps[:, j, :], vcm[:, :, t * 128:(t + 1) * 128],
                                         s2, start=(j == 0), stop=(j == 1),
                                         perf_mode=DR)
                    for j in range(2):
                        t = g * 2 + j
                        nc.vector.scalar_tensor_tensor(
                            out=add1[:, t, :], in0=ps[:, j, :],
                            scalar=sc_att[:, t:t + 1], in1=xt[:, t, :],
                            op0=OP.mult, op1=OP.add)
                ln_chunk(add1, list(range(NTH)), rs2, G * 4, 4, "ln2")
                cm_group(add1, rs2, x2n16, x2cm, G, "m")

            # ================= MLP 1 =================
            def mlp(resid, xcm16, w_a, w_b, out_tile, final_dma, post_group=None):
                """out = resid + W_b.T @ gelu(W_a.T @ xcm16) / SW.
                post_group(G) is emitted after each 4-tile out group."""
                h8 = ap.tile([128, FT, T], F8, tag="h8")
                for ch in range(4):
                    for ft in range(FT):
                        ps = pmm.tile([128, 512], F32, tag="mm")
                        for kt in range(CT):
                            nc.tensor.matmul(
                                ps, w_a[:, kt, ft * 128:(ft + 1) * 128],
                                xcm16[:, kt, ch * 512:(ch + 1) * 512],
                                start=(kt == 0), stop=(kt == CT - 1))
                        with nc.allow_low_precision(reason="fp8 h"):
                            nc.scalar.activation(h8[:, ft, ch * 512:(ch + 1) * 512],
                                                 ps, AF.Gelu, scale=1.0 / SW)
                for G in range(4):
                    for sg in range(2):
                        g = G * 2 + sg
                        ps = pmm.tile([128, 2, C], F32, tag="mm")
                        for j in range(2):
                            t = g * 2 + j
                            for fp in range(FT // 2):
                                nc.tensor.matmul(
                                    ps[:, j, :],
                                    h8[:, 2 * fp:2 * fp + 2, t * 128:(t + 1) * 128],
                                    w_b[:, 2 * fp:2 * fp + 2, :],
                                    start=(fp == 0 and j == 0),
                                    stop=(fp == FT // 2 - 1 and j == 1),
                                    perf_mode=DR)
                        if final_dma:
                            ot = tp.tile([128, 2, C], F32, tag="out_sb", bufs=4)
                            nc.vector.scalar_tensor_tensor(
                                out=ot, in0=ps, scalar=1.0 / SW,
                                in1=resid[:, g * 2:g * 2 + 2, :],
                                op0=OP.mult, op1=OP.add)
                            nc.sync.dma_start(
                                out=io["y"][:, :].rearrange(
                                    "(tt p) c -> p tt c", p=128)[:, g * 2:g * 2 + 2, :],
                                in_=ot)
                        else:
                            nc.vector.scalar_tensor_tensor(
                                out=out_tile[:, g * 2:g * 2 + 2, :], in0=ps,
                                scalar=1.0 / SW, in1=resid[:, g * 2:g * 2 + 2, :],
                                op0=OP.mult, op1=OP.add)
                    if post_group is not None:
                        post_group(G)
                return out_tile

            rs2 = ln_rs(add1, list(range(NTH)), "ln2", chunk=4)
            dump("d_rs2", rs2)
            add2 = ap.tile([128, NTH, C], F32, tag="residB")
            rs3 = stg.tile([128, NTH], F32, tag="rs_ln3")
            x3n16, n3cm = cm_tiles("ca")

            def post_mlp1(G):
                ln_chunk(add2, list(range(NTH)), rs3, G * 4, 4, "ln3")
                cm_group(add2, rs3, x3n16, n3cm, G, "ca")

            mlp(add1, x2cm, w1, w2, add2, False, post_group=post_mlp1)

            # ================= ChannelAttention =================
            dump("d_add2", add2[:, 0, :])
            qkw = wload("qk8", CT, 2 * C)
            vvw = wload("vv8", CT, C)
            pjw = wload("pj8", CT, C)
            bdm = wp.tile([128, CT, 128], BF16, tag="bdm")
            nc.sync.dma_start(out=bdm, in_=io["bdmask"][:, :, :])
            temp_sb = wp.tile([128, CT], F32, tag="temp")
            nc.sync.dma_start(out=temp_sb, in_=io["temp_c"][:, :])


            # qk projections (fp8 x SQK); gram + norms accumulated on PE
            qkt8 = ap.tile([128, NTH, 2 * C], F8, tag="qkt8")
            ps_a0 = pacc.tile([128, C], F32, tag="ps_s0")
            ps_a1 = pacc.tile([128, C], F32, tag="ps_s1")
            ps_nrm = pacc.tile([1, 2 * C], F32, tag="ps_nrm")
            sq16 = ap.tile([128, NTH, 2 * C], BF16, tag="sq16")
            for p in range(NTH // 2):
                for i in range(2):
                    t = 2 * p + i
                    ps = pmm.tile([128, 2 * C], F32, tag="mm")
                    for kt in range(CT):
                        nc.tensor.matmul(ps, n3cm[:, kt, t * 128:(t + 1) * 128],
                                         qkw[:, kt, :], start=(kt == 0),
                                         stop=(kt == CT - 1))
                    with nc.allow_low_precision(reason="fp8 qk"):
                        nc.scalar.activation(qkt8[:, t, :], ps, AF.Identity)
                        nc.vector.tensor_tensor(out=sq16[:, t, :],
                                                in0=qkt8[:, t, :],
                                                in1=qkt8[:, t, :], op=OP.mult)
                st_, sp_ = (p == 0), (p == NTH // 2 - 1)
                nc.tensor.matmul(ps_nrm, ones8, sq16[:, 2 * p, :],
                                 start=st_, stop=False)
                nc.tensor.matmul(ps_nrm, ones8, sq16[:, 2 * p + 1, :],
                                 start=False, stop=sp_)
                nc.tensor.matmul(ps_a0, qkt8[:, 2 * p:2 * p + 2, 0:128],
                                 qkt8[:, 2 * p:2 * p + 2, C:2 * C],
                                 start=st_, stop=sp_, perf_mode=DR)
                nc.tensor.matmul(ps_a1, qkt8[:, 2 * p:2 * p + 2, 128:C],
                                 qkt8[:, 2 * p:2 * p + 2, C:2 * C],
                                 start=st_, stop=sp_, perf_mode=DR)

            dump("d_qkt", qkt8[:, 0, :])

            # pack the used per-head diagonal 32x32 gram blocks + norms
            ca_tx = stg.tile([128, CCW], F32, tag="ca_tx")
            for hh in range(H_CH):
                ct, r0 = hh // 4, (hh % 4) * HD
                src_ps = ps_a0 if ct == 0 else ps_a1
                nc.vector.tensor_copy(ca_tx[r0:r0 + HD, ct * HD:(ct + 1) * HD],
                                      src_ps[r0:r0 + HD, hh * HD:(hh + 1) * HD])
            nrm_sb = stg.tile([1, 2 * C], F32, tag="nrm_sb")
            nc.vector.tensor_copy(nrm_sb, ps_nrm)
            ps_fl = pmm.tile([128, 2 * CT], F32, tag="mm", name="ps_fl")
            for i in range(2 * CT):
                nc.tensor.matmul(ps_fl[:, i:i + 1],
                                 nrm_sb[0:1, i * 128:(i + 1) * 128],
                                 ident32[0:1, 0:1], is_transpose=True,
                                 start=(i == 0), stop=(i == 2 * CT - 1))
            nc.vector.tensor_copy(ca_tx[:, 2 * HD:CCW], ps_fl)
            nc.sync.dma_start(out=cc_in[:, :], in_=ca_tx[:, :])
            nc.gpsimd.collective_compute(
                "AllGather", OP.bypass, replica_groups=REPLICA_GROUPS,
                ins=[cc_in[:, :]], outs=[cc_out[:, :, :]])

            # MLP2 weights arrive during the collective
            w3 = wload("w3_8", CT, DFF, tag="wmlp_a")
            w4 = wload("w4_8", FT, C, tag="wmlp_b")

            # v channel-major (overlaps the collective)
            vcm3 = ap.tile([128, CT, T], F8, tag="vcm")
            for ch in range(4):
                for ct in range(CT):
                    ps = pmm.tile([128, 512], F32, tag="mm")
                    for kt in range(CT):
                        nc.tensor.matmul(ps, vvw[:, kt, ct * 128:(ct + 1) * 128],
                                         n3cm[:, kt, ch * 512:(ch + 1) * 512],
                                         start=(kt == 0), stop=(kt == CT - 1))
                    with nc.allow_low_precision(reason="fp8 v"):
                        nc.scalar.activation(vcm3[:, ct, ch * 512:(ch + 1) * 512],
                                             ps, AF.Identity)


            # ---- post-collective epilogue ----
            ca_rx = stg.tile([128, 2, CCW], F32, tag="ca_rx")
            nc.sync.dma_start(out=ca_rx, in_=cc_out[:, :, :].rearrange("r p w -> p r w"))
            ca_tot = stg.tile([128, CCW], F32, tag="ca_tot")
            nc.vector.tensor_tensor(out=ca_tot, in0=ca_rx[:, 0, :],
                                    in1=ca_rx[:, 1, :], op=OP.add)

            dump("d_catot", ca_tot)
            nktot = ca_tot[:, 2 * HD:CCW]    # [128, 4]: qsumsq-cols | ksumsq-cols
            # inv norms via Newton rsqrt (columns, f32)
            invn = stg.tile([128, 2 * CT], F32, tag="invn")
            nw_t = tp.tile([128, 2 * CT], F32, tag="nw_t")
            nc.vector.tensor_scalar(out=invn.bitcast(I32), in0=nktot.bitcast(I32),
                                    scalar1=1, scalar2=None,
                                    op0=OP.logical_shift_right)
            nc.vector.tensor_scalar(out=invn.bitcast(I32), in0=invn.bitcast(I32),
                                    scalar1=-1, scalar2=MAGIC,
                                    op0=OP.mult, op1=OP.add)
            for _ in range(2):
                nc.vector.tensor_tensor(out=nw_t, in0=invn, in1=invn, op=OP.mult)
                nc.vector.tensor_tensor(out=nw_t, in0=nw_t, in1=nktot, op=OP.mult)
                nc.vector.tensor_scalar(out=nw_t, in0=nw_t, scalar1=-0.5,
                                        scalar2=1.5, op0=OP.mult, op1=OP.add)
                nc.vector.tensor_tensor(out=invn, in0=invn, in1=nw_t, op=OP.mult)
            # scale invq by temperature and the poly-exp 1/4 folding
            invq = stg.tile([128, CT], F32, tag="invq")
            nc.vector.tensor_tensor(out=invq, in0=invn[:, 0:CT], in1=temp_sb,
                                    op=OP.mult)
            nc.vector.tensor_scalar(out=invq, in0=invq, scalar1=0.25,
                                    scalar2=None, op0=OP.mult)
            # invk back to a row [1, C] via PE transpose, broadcast to [128, C]
            ps_kf = pmm.tile([1, C], F32, tag="mm", name="ps_kf")
            for ct in range(CT):
                nc.tensor.matmul(ps_kf[0:1, ct * 128:(ct + 1) * 128],
                                 invn[:, CT + ct:CT + ct + 1],
                                 ident32, is_transpose=True,
                                 start=(ct == 0), stop=(ct == 1))
            invk_row = tp.tile([1, C], F32, tag="invk_row")
            nc.vector.tensor_copy(invk_row, ps_kf)
            ones_row16 = tp.tile([1, 128], BF16, tag="ones_row16")
            nc.vector.memset(ones_row16, 1.0)
            invk_row16 = tp.tile([1, C], BF16, tag="invk_row16")
            with nc.allow_low_precision(reason="bf16 bcast operand"):
                nc.vector.tensor_copy(invk_row16, invk_row)
            ps_bk = pmm.tile([128, C], F32, tag="mm", name="ps_bk")
            nc.tensor.matmul(ps_bk, ones_row16, invk_row16, start=True, stop=True)
            bk = stg.tile([128, C], F32, tag="bk")
            nc.vector.tensor_copy(bk, ps_bk)

            # logits = gram * invq(part) * invk(elem); per-head blocks only
            attn_l = stg.tile([128, CT, 128], F32, tag="attn_l")
            nc.vector.memset(attn_l, 0.0)
            for hh in range(H_CH):
                ct, r0 = hh // 4, (hh % 4) * HD
                nc.vector.scalar_tensor_tensor(
                    out=attn_l[r0:r0 + HD, ct, r0:r0 + HD],
                    in0=ca_tot[r0:r0 + HD, ct * HD:(ct + 1) * HD],
                    scalar=invq[r0:r0 + HD, ct:ct + 1],
                    in1=bk[r0:r0 + HD, hh * HD:(hh + 1) * HD],
                    op0=OP.mult, op1=OP.mult)

            # exp via (1 + u + u^2/2 + u^3/6)^4; u = logits/4 folded into invq
            u = attn_l
            pe_ = stg.tile([128, CT, 128], F32, tag="attn_p")
            nc.vector.tensor_scalar(out=pe_, in0=u, scalar1=1.0 / 6.0,
                                    scalar2=0.5, op0=OP.mult, op1=OP.add)
            nc.vector.tensor_tensor(out=pe_, in0=pe_, in1=u, op=OP.mult)
            nc.vector.tensor_scalar(out=pe_, in0=pe_, scalar1=1.0, scalar2=None,
                                    op0=OP.add)
            nc.vector.tensor_tensor(out=pe_, in0=pe_, in1=u, op=OP.mult)
            nc.vector.tensor_scalar(out=pe_, in0=pe_, scalar1=1.0, scalar2=None,
                                    op0=OP.add)
            nc.vector.tensor_tensor(out=pe_, in0=pe_, in1=pe_, op=OP.mult)
            nc.vector.tensor_tensor(out=pe_, in0=pe_, in1=pe_, op=OP.mult)
            # mask off-block entries, row-normalize
            with nc.allow_low_precision(reason="bf16 mask"):
                nc.vector.tensor_tensor(out=pe_, in0=pe_, in1=bdm, op=OP.mult)
            rsum = tp.tile([128, CT], F32, tag="attn_rs")
            nc.vector.tensor_reduce(rsum.unsqueeze(-1), pe_, axis=AX.X, op=OP.add)
            rinv = tp.tile([128, CT], F32, tag="attn_ri")
            nc.vector.reciprocal(rinv, rsum)
            attn16 = stg.tile([128, CT, 128], BF16, tag="attn16")
            with nc.allow_low_precision(reason="bf16 attn"):
                nc.vector.tensor_tensor(out=attn16, in0=pe_,
                                        in1=bscale(rinv, 0, CT, 128), op=OP.mult)

            dump("d_attn", attn16)

            # transpose attn blocks -> lhsT; attn @ v ; proj ; add3
            at_bd = stg.tile([128, CT, 128], BF16, tag="at_bd")
            for ct in range(CT):
                ps_at = pmmt.tile([128, 128], BF16, tag="mmt", name=f"ps_at{ct}")
                nc.tensor.transpose(ps_at, attn16[:, ct, :], ident16)
                with nc.allow_low_precision(reason="bf16 attn lhsT"):
                    nc.vector.tensor_copy(at_bd[:, ct, :], ps_at)

            cac = ap.tile([128, CT, T], F8, tag="cac")
            for ct in range(CT):
                for ch in range(4):
                    ps = pmm.tile([128, 512], F32, tag="mm")
                    nc.tensor.matmul(ps, at_bd[:, ct, :],
                                     vcm3[:, ct, ch * 512:(ch + 1) * 512],
                                     start=True, stop=True)
                    with nc.allow_low_precision(reason="fp8 cac"):
                        nc.scalar.activation(cac[:, ct, ch * 512:(ch + 1) * 512],
                                             ps, AF.Identity)

            add3 = ap.tile([128, NTH, C], F32, tag="residA")
            rs4 = stg.tile([128, NTH], F32, tag="rs_ln4")
            x4n16, x4cm = cm_tiles("f")
            for G in range(4):
                for sg in range(2):
                    g = G * 2 + sg
                    ps = pmm.tile([128, 2, C], F32, tag="mm")
                    for j in range(2):
                        t = g * 2 + j
                        nc.tensor.matmul(ps[:, j, :], cac[:, :, t * 128:(t + 1) * 128],
                                         pjw, start=(j == 0), stop=(j == 1),
                                         perf_mode=DR)
                    nc.vector.scalar_tensor_tensor(
                        out=add3[:, g * 2:g * 2 + 2, :], in0=ps,
                        scalar=1.0 / (SW * SW), in1=add2[:, g * 2:g * 2 + 2, :],
                        op0=OP.mult, op1=OP.add)
                ln_chunk(add3, list(range(NTH)), rs4, G * 4, 4, "ln4")
                cm_group(add3, rs4, x4n16, x4cm, G, "f")

            # ================= MLP 2 (writes y) =================
            mlp(add3, x4cm, w3, w4, None, True)

    nc.compile()
    _CACHE["nc"] = nc
    return nc


def prep_host(inputs):
    """Weight/layout prep shared by all cores (no arithmetic on x)."""
    f8 = ml_dtypes.float8_e4m3
    f = lambda k: np.asarray(inputs[k], np.float32)
    for k in ("ln1_b", "ln2_b", "ln3_b", "ln4_b", "m1_b1", "m1_b2",
              "m2_b1", "m2_b2", "proj_b"):
        assert np.abs(f(k)).max() == 0.0, f"{k} nonzero; bias path not emitted"
    g1, g2, g3, g4 = f("ln1_g"), f("ln2_g"), f("ln3_g"), f("ln4_g")

    def fold(w, g, center=True):
        """M = Pc @ (diag(g) @ w.T): [in, out] with LN gamma + centering."""
        m = (w * g[None, :]).T.astype(np.float32)
        if center:
            m = m - m.mean(axis=0, keepdims=True)
        return m

    def arr_ct(m, scale):
        """[C_in, O] -> [128, C_in//128, O] fp8 with scale."""
        ci, o = m.shape
        return np.ascontiguousarray(
            (m * scale).reshape(ci // 128, 128, o).transpose(1, 0, 2)
        ).astype(f8)

    qkv_w = f("qkv_w")
    wkq = np.concatenate([fold(f("wk"), g1), fold(f("wq"), g1)], axis=1)
    qk_ca = fold(qkv_w[:2 * C], g3)          # [C, 2C]
    vv_ca = fold(qkv_w[2 * C:], g3)          # [C, C]
    w1 = fold(f("m1_w1"), g2)
    w3 = fold(f("m2_w1"), g4)
    w2 = f("m1_w2").T                        # [DFF, C]
    w4 = f("m2_w2").T
    wr = f("wr").T                           # [C, C]
    pj = f("proj_w").T

    bdmask = np.zeros((128, CT, 128), np.float32)
    for hh in range(H_CH):
        ct, r0 = hh // 4, (hh % 4) * HD
        bdmask[r0:r0 + HD, ct, r0:r0 + HD] = 1.0

    temp = np.repeat(f("temperature").reshape(H_CH), HD).reshape(CT, 128).T

    return {
        "wkq8": arr_ct(wkq, SW),
        "wv8": arr_ct(fold(f("wv"), g1), SW),
        "wr16": np.ascontiguousarray(
            (wr * SWR).reshape(CT, 128, C).transpose(1, 0, 2)
        ).astype(ml_dtypes.bfloat16),
        "qk8": arr_ct(qk_ca, SQK),
        "vv8": arr_ct(vv_ca, SW),
        "pj8": arr_ct(pj, SW),
        "w1_8": arr_ct(w1, SW),
        "w2_8": arr_ct(w2, SW),
        "w3_8": arr_ct(w3, SW),
        "w4_8": arr_ct(w4, SW),
        "ident8": np.eye(128, dtype=f8),
        "ident16": np.eye(128, dtype=ml_dtypes.bfloat16),
        "ones8": np.ones((128, 1), f8),
        "bdmask": bdmask.astype(ml_dtypes.bfloat16),
        "temp_c": np.ascontiguousarray(temp, np.float32),
        "ident32": np.eye(128, dtype=np.float32),
    }


def make_in_maps(inputs):
    shared = prep_host(inputs)
    f8 = ml_dtypes.float8_e4m3
    x = np.asarray(inputs["x"], np.float32)
    in_maps = []
    for c in range(NCORES):
        b, hlf = c // 2, c % 2
        # own half first, peer half second (layout only, no arithmetic)
        xo = np.concatenate([x[b, hlf * T:(hlf + 1) * T, :],
                             x[b, (1 - hlf) * T:(2 - hlf) * T, :]], axis=0)
        m = dict(shared)
        m["xt16"] = np.ascontiguousarray(
            xo.reshape(NT, 128, C).transpose(1, 0, 2)).astype(ml_dtypes.bfloat16)
        m["xcm8"] = np.ascontiguousarray(
            xo.T.reshape(CT, 128, N).transpose(1, 0, 2)).astype(f8)
        in_maps.append(m)
    return in_maps


def assemble(results):
    y = np.empty((B, N, C), np.float32)
    for c in range(NCORES):
        b, hlf = c // 2, c % 2
        y[b, hlf * T:(hlf + 1) * T, :] = results[c]["y"]
    return y


def kernel(**inputs):
    from concourse.bass_utils import run_bass_kernel_spmd

    nc = build_program()
    in_maps = make_in_maps(inputs)
    res = run_bass_kernel_spmd(nc, in_maps, list(range(NCORES)))
    return assemble(res.results)


# revision 24
# speedup vs baseline: 1.0280x; 1.0042x over previous
"""DualTransformerBlock Trainium2 kernel (v2).

Strategy (8 cores: core c -> sample b=c//2, token half h=c%2, T=2048):
  - EfficientAttention reassociated: att = V @ ((K^T Q)/colsum @ wr.T); the
    [N,N] context never materializes.  The K/Q-side stats (exp projections and
    the [C,C] S matrix) are computed REDUNDANTLY for all 4096 tokens on both
    cores of a pair, so no collective is needed for EA.
  - Each core's x tensors are ordered with ITS half first (tiles 0..15 = own
    half, 16..31 = peer half); the S/stat sums are order-invariant, so one
    SPMD program serves all cores.
  - ChannelAttention still needs one cross-half reduction (per-head gram +
    q/k norms); exchanged via AllGather (cheaper than AllReduce in the cost
    model) and summed locally.
  - LayerNorm: mean-centering is folded into the following weight matrices on
    the host ((x-mean) @ W == x @ (Pc W)); the per-token 1/sigma comes from a
    bit-trick Newton rsqrt (no activation-table use) and is fused into Act
    scale= or evacuation scalars.
  - Heavy matmuls run in fp8e4m3 with DoubleRow perf mode (2 contraction
    tiles per instruction, 0.5 cyc/row).  Host-side exponent scaling keeps
    fp8 operands out of the subnormal range; scales cancel or fold into
    per-token evacuation scalars.
  - Activation tables: only Exp (EA) and Gelu (MLPs) -> 2 table loads.  The
    tiny per-head channel-attn softmax uses a polynomial exp on DVE/Pool.
"""

import os
import sys

sys.path.insert(0, "/opt/trn_rl_repo")

import numpy as np
import ml_dtypes

import concourse.bass as bass
import concourse.mybir as mybir
from concourse import bacc
from concourse.tile import TileContext

F32 = mybir.dt.float32
BF16 = mybir.dt.bfloat16
F8 = mybir.dt.float8e4
I32 = mybir.dt.int32
AF = mybir.ActivationFunctionType
OP = mybir.AluOpType
AX = mybir.AxisListType
DR = mybir.MatmulPerfMode.DoubleRow

B, N, C = 4, 4096, 256
H_CH = 8
HD = C // H_CH          # 32
DFF = 4 * C             # 1024
EPS_LN = 1e-5

NCORES = 8
T = N // 2              # 2048 tokens per core half
NT = N // 128           # 32 token tiles (full sample)
NTH = T // 128          # 16 token tiles (own half)
CT = C // 128           # 2 channel tiles
FT = DFF // 128         # 8 ff tiles
REPLICA_GROUPS = [[0, 1], [2, 3], [4, 5], [6, 7]]
CCW = 2 * HD + 2 * CT   # collective payload width (68)

# fp8 exponent scales (host folds these into weights; device descales)
SW = 32.0       # generic weight scale (wkq, wv, w1..w4, qkv-v, proj)
SQK = 8.0       # channel-attn qk scale
SK8 = 64.0      # normalized-k softmax scale
SWR = 256.0     # wr scale
MAGIC = 0x5F3759DF

_CACHE = {}


def build_program():
    if "nc" in _CACHE:
        return _CACHE["nc"]
    nc = bacc.Bacc(None, target_bir_lowering=False)

    io = {}

    def param(name, shape, dt=F32):
        io[name] = nc.declare_dram_parameter(name, list(shape), dt, isOutput=False)

    param("xt16", (128, NT, C), BF16)       # token-major x, all tokens, own half first
    param("xcm8", (128, CT, N), F8)         # channel-major x, all tokens
    param("wkq8", (128, CT, 2 * C), F8)     # [K|Q] proj  (centered, g1, xSW)
    param("wv8", (128, CT, C), F8)          # EA values proj (centered, g1, xSW)
    param("wr16", (128, CT, C), BF16)       # EA out proj (xSWR)
    param("qk8", (128, CT, 2 * C), F8)      # CA [q|k] proj (centered, g3, xSQK)
    param("vv8", (128, CT, C), F8)          # CA v proj (centered, g3, xSW)
    param("pj8", (128, CT, C), F8)          # CA out proj (xSW)
    param("w1_8", (128, CT, DFF), F8)       # MLP1 fc1 (centered, g2, xSW)
    param("w2_8", (128, FT, C), F8)         # MLP1 fc2 (xSW)
    param("w3_8", (128, CT, DFF), F8)       # MLP2 fc1 (centered, g4, xSW)
    param("w4_8", (128, FT, C), F8)         # MLP2 fc2 (xSW)
    param("ident8", (128, 128), F8)
    param("ident16", (128, 128), BF16)
    param("ident32", (128, 128), F32)
    param("ones8", (128, 1), F8)
    param("bdmask", (128, CT, 128), BF16)   # per-head 32x32 block-diag mask
    param("temp_c", (128, CT), F32)         # CA temperature, column layout
    io["y"] = nc.declare_dram_parameter("y", [T, C], F32, isOutput=True)
    DBG = os.environ.get("KDBG", "0") == "1"
    dbg_specs = {
        "d_rs1": (128, NT), "d_kq": (128, 2 * C), "d_st": (128, CT, C),
        "d_csum": (128, CT), "d_s2": (128, CT, C), "d_add1": (128, C),
        "d_vcm": (128, CT, 512), "d_add2": (128, C), "d_qkt": (128, 2 * C),
        "d_catot": (128, CCW), "d_attn": (128, CT, 128), "d_add3": (128, C),
        "d_rs2": (128, NTH),
    }
    if DBG:
        for k, shp in dbg_specs.items():
            io[k] = nc.declare_dram_parameter(k, list(shp), F32, isOutput=True)

    cc_in = nc.dram_tensor("cc_in", [128, CCW], F32)
    cc_out = nc.dram_tensor("cc_out", [2, 128, CCW], F32)

    with TileContext(nc) as tc:
        with (
            tc.tile_pool(name="wpool", bufs=1) as wp,
            tc.tile_pool(name="apool", bufs=1) as ap,
            tc.tile_pool(name="tmp", bufs=3) as tp,
            tc.tile_pool(name="stage", bufs=1) as stg,
            tc.tile_pool(name="pacc", bufs=1, space="PSUM") as pacc,
            tc.tile_pool(name="pmm", bufs=3, space="PSUM") as pmm,
            tc.tile_pool(name="pmmt", bufs=2, space="PSUM") as pmmt,
        ):
            # ---------------- input DMA ----------------
            xt = ap.tile([128, NT, C], BF16, tag="xt")
            for g in range(4):
                nc.sync.dma_start(out=xt[:, g * 8:(g + 1) * 8, :],
                                  in_=io["xt16"][:, g * 8:(g + 1) * 8, :])
            xcm = ap.tile([128, CT, N], F8, tag="xcm")
            for g in range(4):
                nc.sync.dma_start(
                    out=xcm[:, :, g * (N // 4):(g + 1) * (N // 4)],
                    in_=io["xcm8"][:, :, g * (N // 4):(g + 1) * (N // 4)])

            def wload(name, d1, d2, dt=F8, tag=None):
                t = wp.tile([128, d1, d2], dt, tag=tag or name)
                nc.sync.dma_start(out=t, in_=io[name][:, :, :])
                return t

            wkq = wload("wkq8", CT, 2 * C)
            wv = wload("wv8", CT, C)
            wr16 = wload("wr16", CT, C, BF16)
            w1 = wload("w1_8", CT, DFF, tag="wmlp_a")
            w2 = wload("w2_8", FT, C, tag="wmlp_b")
            ident8 = wp.tile([128, 128], F8, tag="ident8")
            nc.sync.dma_start(out=ident8, in_=io["ident8"][:, :])
            ident16 = wp.tile([128, 128], BF16, tag="ident16")
            nc.sync.dma_start(out=ident16, in_=io["ident16"][:, :])
            ident32 = wp.tile([128, 128], F32, tag="ident32")
            nc.sync.dma_start(out=ident32, in_=io["ident32"][:, :])
            ones8 = wp.tile([128, 1], F8, tag="ones8")
            nc.sync.dma_start(out=ones8, in_=io["ones8"][:, :])

            def dump(name, src_ap):
                if DBG:
                    dt = stg.tile(list(src_ap.shape), F32, tag=f"dbg_{name}",
                                  name=f"dbg_{name}", bufs=1)
                    nc.vector.tensor_copy(dt, src_ap)
                    nc.sync.dma_start(out=io[name].__getitem__(
                        tuple([slice(None)] * len(src_ap.shape))), in_=dt)

            # ---------------- helpers ----------------
            def ln_chunk(src_t, tiles, y, c0, m, tag, div=1.0):
                """Write rsqrt(var+eps)/div for tiles[c0:c0+m] into y[:, c0:c0+m]."""
                mvg = tp.tile([128, m, 2], F32, tag=f"mvg_{tag}",
                              name=f"mvg_{tag}_{c0}", bufs=2)
                for j in range(m):
                    st6 = tp.tile([128, 6], BF16, tag="st6", bufs=4)
                    nc.vector.bn_stats(out=st6, in_=src_t[:, tiles[c0 + j], :])
                    nc.vector.bn_aggr(out=mvg[:, j, :], in_=st6)
                vpe = tp.tile([128, m], F32, tag=f"vpe_{tag}",
                              name=f"vpe_{tag}_{c0}", bufs=2)
                nc.vector.tensor_scalar(out=vpe, in0=mvg[:, :, 1],
                                        scalar1=EPS_LN, scalar2=None,
                                        op0=OP.add)
                ys = y[:, c0:c0 + m]
                nc.vector.tensor_scalar(out=ys.bitcast(I32),
                                        in0=vpe.bitcast(I32),
                                        scalar1=1, scalar2=None,
                                        op0=OP.logical_shift_right)
                nc.vector.tensor_scalar(out=ys.bitcast(I32),
                                        in0=ys.bitcast(I32),
                                        scalar1=-1, scalar2=MAGIC,
                                        op0=OP.mult, op1=OP.add)
                t_ = tp.tile([128, m], F32, tag=f"nt_{tag}",
                             name=f"nt_{tag}_{c0}", bufs=2)
                for it in range(2):
                    last = it == 1
                    nc.gpsimd.tensor_tensor(out=t_, in0=ys, in1=ys, op=OP.mult)
                    nc.gpsimd.tensor_tensor(out=t_, in0=t_, in1=vpe, op=OP.mult)
                    nc.vector.tensor_scalar(
                        out=t_, in0=t_,
                        scalar1=(-0.5 / div) if last else -0.5,
                        scalar2=(1.5 / div) if last else 1.5,
                        op0=OP.mult, op1=OP.add)
                    nc.gpsimd.tensor_tensor(out=ys, in0=ys, in1=t_, op=OP.mult)

            def ln_rs(src_t, tiles, tag, div=1.0, chunk=8):
                n = len(tiles)
                y = stg.tile([128, n], F32, tag=f"rs_{tag}", name=f"rs_{tag}")
                for c0 in range(0, n, chunk):
                    ln_chunk(src_t, tiles, y, c0, min(chunk, n - c0), tag, div)
                return y

            def bscale(sc, j0, n, width=C):
                """[128, n] slice of sc -> stride-0 broadcast [128, n, width]."""
                return sc[:, j0:j0 + n].unsqueeze(-1).broadcast_to((128, n, width))

            def cm_group(src_t, rs, x16, cm, g, tag):
                """tiles 4g..4g+4: mult (alt Pool/DVE) + DMA xbar transposes."""
                with nc.allow_low_precision(reason="bf16 matmul operand"):
                    eng = nc.gpsimd if g % 2 == 0 else nc.vector
                    eng.tensor_tensor(
                        out=x16[:, g * 4:(g + 1) * 4, :],
                        in0=src_t[:, g * 4:(g + 1) * 4, :],
                        in1=bscale(rs, g * 4, 4), op=OP.mult)
                    for t in range(g * 4, g * 4 + 4):
                        nc.sync.dma_start_transpose(
                            cm[:, :, t * 128:(t + 1) * 128], x16[:, t, :])

            def cm_tiles(tag):
                x16 = ap.tile([128, NTH, C], BF16, tag=f"tm_{tag}",
                              name=f"tm_{tag}")
                cm = ap.tile([128, CT, T], BF16, tag=f"cm_{tag}",
                             name=f"cm_{tag}")
                return x16, cm

            # ================= EfficientAttention =================
            # ================= EfficientAttention =================
            # ================= EfficientAttention =================
            # sc_kq = rs1/SW for ALL 32 tiles (stats replicated across pair)
            sc_kq = ln_rs(xt, list(range(NT)), "ln1", div=SW)

            kq8 = ap.tile([128, NT, 2 * C], F8, tag="kq8")
            ksums = stg.tile([128, NT], F32, tag="ksums")
            rinv64 = stg.tile([128, NT], F32, tag="rinv64")
            ps_s0 = pacc.tile([128, C], F32, tag="ps_s0")
            ps_s1 = pacc.tile([128, C], F32, tag="ps_s1")

            for p in range(NT // 2):
                for i in range(2):
                    t = 2 * p + i
                    ps = pmm.tile([128, 2 * C], F32, tag="mm")
                    nc.tensor.matmul(ps, xcm[:, :, t * 128:(t + 1) * 128],
                                     wkq, start=True, stop=True, perf_mode=DR)
                    with nc.allow_low_precision(reason="fp8 exp"):
                        nc.scalar.activation(kq8[:, t, :], ps, AF.Exp,
                                             scale=sc_kq[:, t:t + 1])
                # ksum + SK8/ksum for the pair
                nc.vector.tensor_reduce(
                    ksums[:, 2 * p:2 * p + 2].unsqueeze(-1),
                    kq8[:, 2 * p:2 * p + 2, 0:C], axis=AX.X, op=OP.add)
                nc.vector.reciprocal(rinv64[:, 2 * p:2 * p + 2],
                                     ksums[:, 2 * p:2 * p + 2])
                nc.vector.tensor_scalar(out=rinv64[:, 2 * p:2 * p + 2],
                                        in0=rinv64[:, 2 * p:2 * p + 2],
                                        scalar1=SK8, scalar2=None, op0=OP.mult)
                with nc.allow_low_precision(reason="fp8 softmax-k scale"):
                    keng = nc.vector if p % 3 == 0 else nc.gpsimd
                    keng.tensor_tensor(
                        out=kq8[:, 2 * p:2 * p + 2, 0:C],
                        in0=kq8[:, 2 * p:2 * p + 2, 0:C],
                        in1=bscale(rinv64, 2 * p, 2), op=OP.mult)
                # S accumulation (DoubleRow over the token-tile pair)
                st_, sp_ = (p == 0), (p == NT // 2 - 1)
                nc.tensor.matmul(ps_s0, kq8[:, 2 * p:2 * p + 2, C:C + 128],
                                 kq8[:, 2 * p:2 * p + 2, 0:C],
                                 start=st_, stop=sp_, perf_mode=DR)
                nc.tensor.matmul(ps_s1, kq8[:, 2 * p:2 * p + 2, C + 128:2 * C],
                                 kq8[:, 2 * p:2 * p + 2, 0:C],
                                 start=st_, stop=sp_, perf_mode=DR)

            dump("d_rs1", sc_kq)
            dump("d_kq", kq8[:, 0, :])

            # V channel-major for own half (fp8, carries SW)
            vcm = ap.tile([128, CT, T], F8, tag="vcm")
            for ch in range(4):
                for ct in range(CT):
                    ps = pmm.tile([128, 512], F32, tag="mm")
                    nc.tensor.matmul(ps, wv[:, :, ct * 128:(ct + 1) * 128],
                                     xcm[:, :, ch * 512:(ch + 1) * 512],
                                     start=True, stop=True, perf_mode=DR)
                    with nc.allow_low_precision(reason="fp8 V"):
                        nc.scalar.activation(vcm[:, ct, ch * 512:(ch + 1) * 512],
                                             ps, AF.Identity)

            # S_T evac (bf16) + column sums via Act accumulate
            s_t = stg.tile([128, CT, C], BF16, tag="s_t")
            csum = stg.tile([128, CT], F32, tag="csum")
            with nc.allow_low_precision(reason="bf16 S"):
                nc.scalar.activation(s_t[:, 0, :], ps_s0, AF.Identity,
                                     accum_out=csum[:, 0:1])
                nc.scalar.activation(s_t[:, 1, :], ps_s1, AF.Identity,
                                     accum_out=csum[:, 1:2])
            dump("d_st", s_t)
            dump("d_csum", csum)
            cinv = stg.tile([128, CT], F32, tag="cinv")
            nc.vector.reciprocal(cinv, csum)
            wrs = stg.tile([128, CT, C], BF16, tag="wrs")
            with nc.allow_low_precision(reason="bf16 wrs"):
                nc.gpsimd.tensor_tensor(out=wrs, in0=wr16,
                                        in1=bscale(cinv, 0, CT), op=OP.mult)
            # S2[dk, o] = sum_dq S_T[dq, dk] * wrs[dq, o]   (fp8 out, x SWR*SK8)
            s2 = stg.tile([128, CT, C], F8, tag="s2")
            for mt in range(CT):
                ps = pmm.tile([128, C], F32, tag="mm")
                nc.tensor.matmul(ps, s_t[:, 0, mt * 128:(mt + 1) * 128],
                                 wrs[:, 0, :], start=True, stop=False)
                nc.tensor.matmul(ps, s_t[:, 1, mt * 128:(mt + 1) * 128],
                                 wrs[:, 1, :], start=False, stop=True)
                with nc.allow_low_precision(reason="fp8 S2"):
                    nc.scalar.activation(s2[:, mt, :], ps, AF.Identity)

            # att = V @ S2 ; add1 = x + att * rs1 / (SW * SWR * SK8)
            sc_att = stg.tile([128, NTH], F32, tag="sc_att")
            nc.vector.tensor_scalar(out=sc_att, in0=sc_kq[:, 0:NTH],
                                    scalar1=1.0 / SWR,
                                    scalar2=None, op0=OP.mult)
            add1 = ap.tile([128, NTH, C], F32, tag="residA")
            rs2 = stg.tile([128, NTH], F32, tag="rs_ln2")
            x2n16, x2cm = cm_tiles("m")
            for G in range(4):
                for sg in range(2):
                    g = G * 2 + sg
                    ps = pmm.tile([128, 2, C], F32, tag="mm")
                    for j in range(2):
                        t = g * 2 + j
                        nc.tensor.matmul(ps[:, j, :], vcm[:, :, t * 128:(t + 1) * 128],
                                         s2, start=(j == 0), stop=(j == 1),
                                         perf_mode=DR)
                    for j in range(2):
                        t = g * 2 + j
                        nc.vector.scalar_tensor_tensor(
                            out=add1[:, t, :], in0=ps[:, j, :],
                            scalar=sc_att[:, t:t + 1], in1=xt[:, t, :],
                            op0=OP.mult, op1=OP.add)
                ln_chunk(add1, list(range(NTH)), rs2, G * 4, 4, "ln2")
                cm_group(add1, rs2, x2n16, x2cm, G, "m")

            # ================= MLP 1 =================
            def mlp(resid, xcm16, w_a, w_b, out_tile, final_dma, post_group=None):
                """out = resid + W_b.T @ gelu(W_a.T @ xcm16) / SW.
                post_group(G) is emitted after each 4-tile out group."""
                h8 = ap.tile([128, FT, T], F8, tag="h8")
                for ch in range(4):
                    for ft in range(FT):
                        ps = pmm.tile([128, 512], F32, tag="mm")
                        for kt in range(CT):
                            nc.tensor.matmul(
                                ps, w_a[:, kt, ft * 128:(ft + 1) * 128],
                                xcm16[:, kt, ch * 512:(ch + 1) * 512],
                                start=(kt == 0), stop=(kt == CT - 1))
                        with nc.allow_low_precision(reason="fp8 h"):
                            nc.scalar.activation(h8[:, ft, ch * 512:(ch + 1) * 512],
                                                 ps, AF.Gelu, scale=1.0 / SW)
                for G in range(4):
                    for sg in range(2):
                        g = G * 2 + sg
                        ps = pmm.tile([128, 2, C], F32, tag="mm")
                        for j in range(2):
                            t = g * 2 + j
                            for fp in range(FT // 2):
                                nc.tensor.matmul(
                                    ps[:, j, :],
                                    h8[:, 2 * fp:2 * fp + 2, t * 128:(t + 1) * 128],
                                    w_b[:, 2 * fp:2 * fp + 2, :],
                                    start=(fp == 0 and j == 0),
                                    stop=(fp == FT // 2 - 1 and j == 1),
                                    perf_mode=DR)
                        if final_dma:
                            ot = tp.tile([128, 2, C], F32, tag="out_sb", bufs=4)
                            nc.vector.scalar_tensor_tensor(
                                out=ot, in0=ps, scalar=1.0 / SW,
                                in1=resid[:, g * 2:g * 2 + 2, :],
                                op0=OP.mult, op1=OP.add)
                            nc.sync.dma_start(
                                out=io["y"][:, :].rearrange(
                                    "(tt p) c -> p tt c", p=128)[:, g * 2:g * 2 + 2, :],
                                in_=ot)
                        else:
                            nc.vector.scalar_tensor_tensor(
                                out=out_tile[:, g * 2:g * 2 + 2, :], in0=ps,
                                scalar=1.0 / SW, in1=resid[:, g * 2:g * 2 + 2, :],
                                op0=OP.mult, op1=OP.add)
                    if post_group is not None:
                        post_group(G)
                return out_tile

            rs2 = ln_rs(add1, list(range(NTH)), "ln2", chunk=4)
            dump("d_rs2", rs2)
            add2 = ap.tile([128, NTH, C], F32, tag="residB")
            rs3 = stg.tile([128, NTH], F32, tag="rs_ln3")
            x3n16, n3cm = cm_tiles("ca")

            def post_mlp1(G):
                ln_chunk(add2, list(range(NTH)), rs3, G * 4, 4, "ln3")
                cm_group(add2, rs3, x3n16, n3cm, G, "ca")

            mlp(add1, x2cm, w1, w2, add2, False, post_group=post_mlp1)

            # ================= ChannelAttention =================
            dump("d_add2", add2[:, 0, :])
            qkw = wload("qk8", CT, 2 * C)
            vvw = wload("vv8", CT, C)
            pjw = wload("pj8", CT, C)
            bdm = wp.tile([128, CT, 128], BF16, tag="bdm")
            nc.sync.dma_start(out=bdm, in_=io["bdmask"][:, :, :])
            temp_sb = wp.tile([128, CT], F32, tag="temp")
            nc.sync.dma_start(out=temp_sb, in_=io["temp_c"][:, :])


            # qk projections (fp8 x SQK); gram + norms accumulated on PE
            qkt8 = ap.tile([128, NTH, 2 * C], F8, tag="qkt8")
            ps_a0 = pacc.tile([128, C], F32, tag="ps_s0")
            ps_a1 = pacc.tile([128, C], F32, tag="ps_s1")
            ps_nrm = pacc.tile([1, 2 * C], F32, tag="ps_nrm")
            sq16 = ap.tile([128, NTH, 2 * C], BF16, tag="sq16")
            for p in range(NTH // 2):
                for i in range(2):
                    t = 2 * p + i
                    ps = pmm.tile([128, 2 * C], F32, tag="mm")
                    for kt in range(CT):
                        nc.tensor.matmul(ps, n3cm[:, kt, t * 128:(t + 1) * 128],
                                         qkw[:, kt, :], start=(kt == 0),
                                         stop=(kt == CT - 1))
                    with nc.allow_low_precision(reason="fp8 qk"):
                        nc.scalar.activation(qkt8[:, t, :], ps, AF.Identity)
                        nc.vector.tensor_tensor(out=sq16[:, t, :],
                                                in0=qkt8[:, t, :],
                                                in1=qkt8[:, t, :], op=OP.mult)
                st_, sp_ = (p == 0), (p == NTH // 2 - 1)
                nc.tensor.matmul(ps_nrm, ones8, sq16[:, 2 * p, :],
                                 start=st_, stop=False)
                nc.tensor.matmul(ps_nrm, ones8, sq16[:, 2 * p + 1, :],
                                 start=False, stop=sp_)
                nc.tensor.matmul(ps_a0, qkt8[:, 2 * p:2 * p + 2, 0:128],
                                 qkt8[:, 2 * p:2 * p + 2, C:2 * C],
                                 start=st_, stop=sp_, perf_mode=DR)
                nc.tensor.matmul(ps_a1, qkt8[:, 2 * p:2 * p + 2, 128:C],
                                 qkt8[:, 2 * p:2 * p + 2, C:2 * C],
                                 start=st_, stop=sp_, perf_mode=DR)

            dump("d_qkt", qkt8[:, 0, :])

            # pack the used per-head diagonal 32x32 gram blocks + norms
            ca_tx = stg.tile([128, CCW], F32, tag="ca_tx")
            for hh in range(H_CH):
                ct, r0 = hh // 4, (hh % 4) * HD
                src_ps = ps_a0 if ct == 0 else ps_a1
                nc.vector.tensor_copy(ca_tx[r0:r0 + HD, ct * HD:(ct + 1) * HD],
                                      src_ps[r0:r0 + HD, hh * HD:(hh + 1) * HD])
            nrm_sb = stg.tile([1, 2 * C], F32, tag="nrm_sb")
            nc.vector.tensor_copy(nrm_sb, ps_nrm)
            ps_fl = pmm.tile([128, 2 * CT], F32, tag="mm", name="ps_fl")
            for i in range(2 * CT):
                nc.tensor.matmul(ps_fl[:, i:i + 1],
                                 nrm_sb[0:1, i * 128:(i + 1) * 128],
                                 ident32[0:1, 0:1], is_transpose=True,
                                 start=(i == 0), stop=(i == 2 * CT - 1))
            nc.vector.tensor_copy(ca_tx[:, 2 * HD:CCW], ps_fl)
            nc.sync.dma_start(out=cc_in[:, :], in_=ca_tx[:, :])
            nc.gpsimd.collective_compute(
                "AllGather", OP.bypass, replica_groups=REPLICA_GROUPS,
                ins=[cc_in[:, :]], outs=[cc_out[:, :, :]])

            # MLP2 weights arrive during the collective
            w3 = wload("w3_8", CT, DFF, tag="wmlp_a")
            w4 = wload("w4_8", FT, C, tag="wmlp_b")

            # v channel-major (overlaps the collective)
            vcm3 = ap.tile([128, CT, T], F8, tag="vcm")
            for ch in range(4):
                for ct in range(CT):
                    ps = pmm.tile([128, 512], F32, tag="mm")
                    for kt in range(CT):
                        nc.tensor.matmul(ps, vvw[:, kt, ct * 128:(ct + 1) * 128],
                                         n3cm[:, kt, ch * 512:(ch + 1) * 512],
                                         start=(kt == 0), stop=(kt == CT - 1))
                    with nc.allow_low_precision(reason="fp8 v"):
                        nc.scalar.activation(vcm3[:, ct, ch * 512:(ch + 1) * 512],
                                             ps, AF.Identity)


            # ---- post-collective epilogue ----
            ca_rx = stg.tile([128, 2, CCW], F32, tag="ca_rx")
            nc.sync.dma_start(out=ca_rx, in_=cc_out[:, :, :].rearrange("r p w -> p r w"))
            ca_tot = stg.tile([128, CCW], F32, tag="ca_tot")
            nc.vector.tensor_tensor(out=ca_tot, in0=ca_rx[:, 0, :],
                                    in1=ca_rx[:, 1, :], op=OP.add)

            dump("d_catot", ca_tot)
            nktot = ca_tot[:, 2 * HD:CCW]    # [128, 4]: qsumsq-cols | ksumsq-cols
            # inv norms via Newton rsqrt (columns, f32)
            invn = stg.tile([128, 2 * CT], F32, tag="invn")
            nw_t = tp.tile([128, 2 * CT], F32, tag="nw_t")
            nc.vector.tensor_scalar(out=invn.bitcast(I32), in0=nktot.bitcast(I32),
                                    scalar1=1, scalar2=None,
                                    op0=OP.logical_shift_right)
            nc.vector.tensor_scalar(out=invn.bitcast(I32), in0=invn.bitcast(I32),
                                    scalar1=-1, scalar2=MAGIC,
                                    op0=OP.mult, op1=OP.add)
            for _ in range(2):
                nc.vector.tensor_tensor(out=nw_t, in0=invn, in1=invn, op=OP.mult)
                nc.vector.tensor_tensor(out=nw_t, in0=nw_t, in1=nktot, op=OP.mult)
                nc.vector.tensor_scalar(out=nw_t, in0=nw_t, scalar1=-0.5,
                                        scalar2=1.5, op0=OP.mult, op1=OP.add)
                nc.vector.tensor_tensor(out=invn, in0=invn, in1=nw_t, op=OP.mult)
            # scale invq by temperature and the poly-exp 1/4 folding
            invq = stg.tile([128, CT], F32, tag="invq")
            nc.vector.tensor_tensor(out=invq, in0=invn[:, 0:CT], in1=temp_sb,
                                    op=OP.mult)
            nc.vector.tensor_scalar(out=invq, in0=invq, scalar1=0.25,
                                    scalar2=None, op0=OP.mult)
            # invk back to a row [1, C] via PE transpose, broadcast to [128, C]
            ps_kf = pmm.tile([1, C], F32, tag="mm", name="ps_kf")
            for ct in range(CT):
                nc.tensor.matmul(ps_kf[0:1, ct * 128:(ct + 1) * 128],
                                 invn[:, CT + ct:CT + ct + 1],
                                 ident32, is_transpose=True,
                                 start=(ct == 0), stop=(ct == 1))
            invk_row = tp.tile([1, C], F32, tag="invk_row")
            nc.vector.tensor_copy(invk_row, ps_kf)
            ones_row16 = tp.tile([1, 128], BF16, tag="ones_row16")
            nc.vector.memset(ones_row16, 1.0)
            invk_row16 = tp.tile([1, C], BF16, tag="invk_row16")
            with nc.allow_low_precision(reason="bf16 bcast operand"):
                nc.vector.tensor_copy(invk_row16, invk_row)
            ps_bk = pmm.tile([128, C], F32, tag="mm", name="ps_bk")
            nc.tensor.matmul(ps_bk, ones_row16, invk_row16, start=True, stop=True)
            bk = stg.tile([128, C], F32, tag="bk")
            nc.vector.tensor_copy(bk, ps_bk)

            # logits = gram * invq(part) * invk(elem); per-head blocks only
            attn_l = stg.tile([128, CT, 128], F32, tag="attn_l")
            nc.vector.memset(attn_l, 0.0)
            for hh in range(H_CH):
                ct, r0 = hh // 4, (hh % 4) * HD
                nc.vector.scalar_tensor_tensor(
                    out=attn_l[r0:r0 + HD, ct, r0:r0 + HD],
                    in0=ca_tot[r0:r0 + HD, ct * HD:(ct + 1) * HD],
                    scalar=invq[r0:r0 + HD, ct:ct + 1],
                    in1=bk[r0:r0 + HD, hh * HD:(hh + 1) * HD],
                    op0=OP.mult, op1=OP.mult)

            # exp via (1 + u + u^2/2 + u^3/6)^4; u = logits/4 folded into invq
            u = attn_l
            pe_ = stg.tile([128, CT, 128], F32, tag="attn_p")
            nc.vector.tensor_scalar(out=pe_, in0=u, scalar1=1.0 / 6.0,
                                    scalar2=0.5, op0=OP.mult, op1=OP.add)
            nc.vector.tensor_tensor(out=pe_, in0=pe_, in1=u, op=OP.mult)
            nc.vector.tensor_scalar(out=pe_, in0=pe_, scalar1=1.0, scalar2=None,
                                    op0=OP.add)
            nc.vector.tensor_tensor(out=pe_, in0=pe_, in1=u, op=OP.mult)
            nc.vector.tensor_scalar(out=pe_, in0=pe_, scalar1=1.0, scalar2=None,
                                    op0=OP.add)
            nc.vector.tensor_tensor(out=pe_, in0=pe_, in1=pe_, op=OP.mult)
            nc.vector.tensor_tensor(out=pe_, in0=pe_, in1=pe_, op=OP.mult)
            # mask off-block entries, row-normalize
            with nc.allow_low_precision(reason="bf16 mask"):
                nc.vector.tensor_tensor(out=pe_, in0=pe_, in1=bdm, op=OP.mult)
            rsum = tp.tile([128, CT], F32, tag="attn_rs")
            nc.vector.tensor_reduce(rsum.unsqueeze(-1), pe_, axis=AX.X, op=OP.add)
            rinv = tp.tile([128, CT], F32, tag="attn_ri")
            nc.vector.reciprocal(rinv, rsum)
            attn16 = stg.tile([128, CT, 128], BF16, tag="attn16")
            with nc.allow_low_precision(reason="bf16 attn"):
                nc.vector.tensor_tensor(out=attn16, in0=pe_,
                                        in1=bscale(rinv, 0, CT, 128), op=OP.mult)

            dump("d_attn", attn16)

            # transpose attn blocks -> lhsT; attn @ v ; proj ; add3
            at_bd = stg.tile([128, CT, 128], BF16, tag="at_bd")
            for ct in range(CT):
                ps_at = pmmt.tile([128, 128], BF16, tag="mmt", name=f"ps_at{ct}")
                nc.tensor.transpose(ps_at, attn16[:, ct, :], ident16)
                with nc.allow_low_precision(reason="bf16 attn lhsT"):
                    nc.vector.tensor_copy(at_bd[:, ct, :], ps_at)

            cac = ap.tile([128, CT, T], F8, tag="cac")
            for ct in range(CT):
                for ch in range(4):
                    ps = pmm.tile([128, 512], F32, tag="mm")
                    nc.tensor.matmul(ps, at_bd[:, ct, :],
                                     vcm3[:, ct, ch * 512:(ch + 1) * 512],
                                     start=True, stop=True)
                    with nc.allow_low_precision(reason="fp8 cac"):
                        nc.scalar.activation(cac[:, ct, ch * 512:(ch + 1) * 512],
                                             ps, AF.Identity)

            add3 = ap.tile([128, NTH, C], F32, tag="residA")
            rs4 = stg.tile([128, NTH], F32, tag="rs_ln4")
            x4n16, x4cm = cm_tiles("f")
            for G in range(4):
                for sg in range(2):
                    g = G * 2 + sg
                    ps = pmm.tile([128, 2, C], F32, tag="mm")
                    for j in range(2):
                        t = g * 2 + j
                        nc.tensor.matmul(ps[:, j, :], cac[:, :, t * 128:(t + 1) * 128],
                                         pjw, start=(j == 0), stop=(j == 1),
                                         perf_mode=DR)
                    nc.vector.scalar_tensor_tensor(
                        out=add3[:, g * 2:g * 2 + 2, :], in0=ps,
                        scalar=1.0 / (SW * SW), in1=add2[:, g * 2:g * 2 + 2, :],
                        op0=OP.mult, op1=OP.add)
                ln_chunk(add3, list(range(NTH)), rs4, G * 4, 4, "ln4")
                cm_group(add3, rs4, x4n16, x4cm, G, "f")

            # ================= MLP 2 (writes y) =================
            mlp(add3, x4cm, w3, w4, None, True)

    nc.compile()
    _CACHE["nc"] = nc
    return nc


def prep_host(inputs):
    """Weight/layout prep shared by all cores (no arithmetic on x)."""
    f8 = ml_dtypes.float8_e4m3
    f = lambda k: np.asarray(inputs[k], np.float32)
    for k in ("ln1_b", "ln2_b", "ln3_b", "ln4_b", "m1_b1", "m1_b2",
              "m2_b1", "m2_b2", "proj_b"):
        assert np.abs(f(k)).max() == 0.0, f"{k} nonzero; bias path not emitted"
    g1, g2, g3, g4 = f("ln1_g"), f("ln2_g"), f("ln3_g"), f("ln4_g")

    def fold(w, g, center=True):
        """M = Pc @ (diag(g) @ w.T): [in, out] with LN gamma + centering."""
        m = (w * g[None, :]).T.astype(np.float32)
        if center:
            m = m - m.mean(axis=0, keepdims=True)
        return m

    def arr_ct(m, scale):
        """[C_in, O] -> [128, C_in//128, O] fp8 with scale."""
        ci, o = m.shape
        return np.ascontiguousarray(
            (m * scale).reshape(ci // 128, 128, o).transpose(1, 0, 2)
        ).astype(f8)

    qkv_w = f("qkv_w")
    wkq = np.concatenate([fold(f("wk"), g1), fold(f("wq"), g1)], axis=1)
    qk_ca = fold(qkv_w[:2 * C], g3)          # [C, 2C]
    vv_ca = fold(qkv_w[2 * C:], g3)          # [C, C]
    w1 = fold(f("m1_w1"), g2)
    w3 = fold(f("m2_w1"), g4)
    w2 = f("m1_w2").T                        # [DFF, C]
    w4 = f("m2_w2").T
    wr = f("wr").T                           # [C, C]
    pj = f("proj_w").T

    bdmask = np.zeros((128, CT, 128), np.float32)
    for hh in range(H_CH):
        ct, r0 = hh // 4, (hh % 4) * HD
        bdmask[r0:r0 + HD, ct, r0:r0 + HD] = 1.0

    temp = np.repeat(f("temperature").reshape(H_CH), HD).reshape(CT, 128).T

    return {
        "wkq8": arr_ct(wkq, SW),
        "wv8": arr_ct(fold(f("wv"), g1), SW),
        "wr16": np.ascontiguousarray(
            (wr * SWR).reshape(CT, 128, C).transpose(1, 0, 2)
        ).astype(ml_dtypes.bfloat16),
        "qk8": arr_ct(qk_ca, SQK),
        "vv8": arr_ct(vv_ca, SW),
        "pj8": arr_ct(pj, SW),
        "w1_8": arr_ct(w1, SW),
        "w2_8": arr_ct(w2, SW),
        "w3_8": arr_ct(w3, SW),
        "w4_8": arr_ct(w4, SW),
        "ident8": np.eye(128, dtype=f8),
        "ident16": np.eye(128, dtype=ml_dtypes.bfloat16),
        "ones8": np.ones((128, 1), f8),
        "bdmask": bdmask.astype(ml_dtypes.bfloat16),
        "temp_c": np.ascontiguousarray(temp, np.float32),
        "ident32": np.eye(128, dtype=np.float32),
    }


def make_in_maps(inputs):
    shared = prep_host(inputs)
    f8 = ml_dtypes.float8_e4m3
    x = np.asarray(inputs["x"], np.float32)
    in_maps = []
    for c in range(NCORES):
        b, hlf = c // 2, c % 2
        # own half first, peer half second (layout only, no arithmetic)
        xo = np.concatenate([x[b, hlf * T:(hlf + 1) * T, :],
                             x[b, (1 - hlf) * T:(2 - hlf) * T, :]], axis=0)
        m = dict(shared)
        m["xt16"] = np.ascontiguousarray(
            xo.reshape(NT, 128, C).transpose(1, 0, 2)).astype(ml_dtypes.bfloat16)
        m["xcm8"] = np.ascontiguousarray(
            xo.T.reshape(CT, 128, N).transpose(1, 0, 2)).astype(f8)
        in_maps.append(m)
    return in_maps


def assemble(results):
    y = np.empty((B, N, C), np.float32)
    for c in range(NCORES):
        b, hlf = c // 2, c % 2
        y[b, hlf * T:(hlf + 1) * T, :] = results[c]["y"]
    return y


def kernel(**inputs):
    from concourse.bass_utils import run_bass_kernel_spmd

    nc = build_program()
    in_maps = make_in_maps(inputs)
    res = run_bass_kernel_spmd(nc, in_maps, list(range(NCORES)))
    return assemble(res.results)


# revision 28
# speedup vs baseline: 1.0426x; 1.0142x over previous
"""DualTransformerBlock Trainium2 kernel (v2).

Strategy (8 cores: core c -> sample b=c//2, token half h=c%2, T=2048):
  - EfficientAttention reassociated: att = V @ ((K^T Q)/colsum @ wr.T); the
    [N,N] context never materializes.  The K/Q-side stats (exp projections and
    the [C,C] S matrix) are computed REDUNDANTLY for all 4096 tokens on both
    cores of a pair, so no collective is needed for EA.
  - Each core's x tensors are ordered with ITS half first (tiles 0..15 = own
    half, 16..31 = peer half); the S/stat sums are order-invariant, so one
    SPMD program serves all cores.
  - ChannelAttention still needs one cross-half reduction (per-head gram +
    q/k norms); exchanged via AllGather (cheaper than AllReduce in the cost
    model) and summed locally.
  - LayerNorm: mean-centering is folded into the following weight matrices on
    the host ((x-mean) @ W == x @ (Pc W)); the per-token 1/sigma comes from a
    bit-trick Newton rsqrt (no activation-table use) and is fused into Act
    scale= or evacuation scalars.
  - Heavy matmuls run in fp8e4m3 with DoubleRow perf mode (2 contraction
    tiles per instruction, 0.5 cyc/row).  Host-side exponent scaling keeps
    fp8 operands out of the subnormal range; scales cancel or fold into
    per-token evacuation scalars.
  - Activation tables: only Exp (EA) and Gelu (MLPs) -> 2 table loads.  The
    tiny per-head channel-attn softmax uses a polynomial exp on DVE/Pool.
"""

import os
import sys

sys.path.insert(0, "/opt/trn_rl_repo")

import numpy as np
import ml_dtypes

import concourse.bass as bass
import concourse.mybir as mybir
from concourse import bacc
from concourse.tile import TileContext

F32 = mybir.dt.float32
BF16 = mybir.dt.bfloat16
F8 = mybir.dt.float8e4
I32 = mybir.dt.int32
AF = mybir.ActivationFunctionType
OP = mybir.AluOpType
AX = mybir.AxisListType
DR = mybir.MatmulPerfMode.DoubleRow

B, N, C = 4, 4096, 256
H_CH = 8
HD = C // H_CH          # 32
DFF = 4 * C             # 1024
EPS_LN = 1e-5

NCORES = 8
T = N // 2              # 2048 tokens per core half
NT = N // 128           # 32 token tiles (full sample)
NTH = T // 128          # 16 token tiles (own half)
CT = C // 128           # 2 channel tiles
FT = DFF // 128         # 8 ff tiles
REPLICA_GROUPS = [[0, 1], [2, 3], [4, 5], [6, 7]]
CCW = 2 * HD + 2 * CT   # collective payload width (68)

# fp8 exponent scales (host folds these into weights; device descales)
SW = 32.0       # generic weight scale (wkq, wv, w1..w4, qkv-v, proj)
SQK = 8.0       # channel-attn qk scale
SK8 = 64.0      # normalized-k softmax scale
SWR = 256.0     # wr scale
MAGIC = 0x5F3759DF

_CACHE = {}


def build_program():
    if "nc" in _CACHE:
        return _CACHE["nc"]
    nc = bacc.Bacc(None, target_bir_lowering=False)

    io = {}

    def param(name, shape, dt=F32):
        io[name] = nc.declare_dram_parameter(name, list(shape), dt, isOutput=False)

    param("xt16", (128, NT, C), BF16)       # token-major x, all tokens, own half first
    param("xcm8", (128, CT, N), F8)         # channel-major x, all tokens
    param("wkq8", (128, CT, 2 * C), F8)     # [K|Q] proj  (centered, g1, xSW)
    param("wv8", (128, CT, C), F8)          # EA values proj (centered, g1, xSW)
    param("wr16", (128, CT, C), BF16)       # EA out proj (xSWR)
    param("qk8", (128, CT, 2 * C), F8)      # CA [q|k] proj (centered, g3, xSQK)
    param("vv8", (128, CT, C), F8)          # CA v proj (centered, g3, xSW)
    param("pj8", (128, CT, C), F8)          # CA out proj (xSW)
    param("w1_8", (128, CT, DFF), F8)       # MLP1 fc1 (centered, g2, xSW)
    param("w2_8", (128, FT, C), F8)         # MLP1 fc2 (xSW)
    param("w3_8", (128, CT, DFF), F8)       # MLP2 fc1 (centered, g4, xSW)
    param("w4_8", (128, FT, C), F8)         # MLP2 fc2 (xSW)
    param("ident8", (128, 128), F8)
    param("ident16", (128, 128), BF16)
    param("ident32", (128, 128), F32)
    param("ones8", (128, 1), F8)
    param("bdmask", (128, CT, 128), BF16)   # per-head 32x32 block-diag mask
    param("temp_c", (128, CT), F32)         # CA temperature, column layout
    io["y"] = nc.declare_dram_parameter("y", [T, C], F32, isOutput=True)
    DBG = os.environ.get("KDBG", "0") == "1"
    dbg_specs = {
        "d_rs1": (128, NT), "d_kq": (128, 2 * C), "d_st": (128, CT, C),
        "d_csum": (128, CT), "d_s2": (128, CT, C), "d_add1": (128, C),
        "d_vcm": (128, CT, 512), "d_add2": (128, C), "d_qkt": (128, 2 * C),
        "d_catot": (128, CCW), "d_attn": (128, CT, 128), "d_add3": (128, C),
        "d_rs2": (128, NTH),
    }
    if DBG:
        for k, shp in dbg_specs.items():
            io[k] = nc.declare_dram_parameter(k, list(shp), F32, isOutput=True)

    cc_in = nc.dram_tensor("cc_in", [128, CCW], F32)
    cc_out = nc.dram_tensor("cc_out", [2, 128, CCW], F32)

    with TileContext(nc) as tc:
        with (
            tc.tile_pool(name="wpool", bufs=1) as wp,
            tc.tile_pool(name="apool", bufs=1) as ap,
            tc.tile_pool(name="tmp", bufs=3) as tp,
            tc.tile_pool(name="stage", bufs=1) as stg,
            tc.tile_pool(name="pacc", bufs=1, space="PSUM") as pacc,
            tc.tile_pool(name="pmm", bufs=3, space="PSUM") as pmm,
            tc.tile_pool(name="pmmt", bufs=2, space="PSUM") as pmmt,
        ):
            # preload the Exp activation table while DMAs run
            dumm = wp.tile([128, 1], F32, tag="dumm")
            nc.vector.memset(dumm, 0.0)
            nc.scalar.activation(dumm, dumm, AF.Exp)

            # ---------------- input DMA ----------------
            xt = ap.tile([128, NT, C], BF16, tag="xt")
            xcm = ap.tile([128, CT, N], F8, tag="xcm")
            for g in range(4):
                nc.sync.dma_start(
                    out=xcm[:, :, g * (N // 4):(g + 1) * (N // 4)],
                    in_=io["xcm8"][:, :, g * (N // 4):(g + 1) * (N // 4)])
                nc.sync.dma_start(out=xt[:, g * 8:(g + 1) * 8, :],
                                  in_=io["xt16"][:, g * 8:(g + 1) * 8, :])

            def wload(name, d1, d2, dt=F8, tag=None):
                t = wp.tile([128, d1, d2], dt, tag=tag or name)
                nc.sync.dma_start(out=t, in_=io[name][:, :, :])
                return t

            wkq = wload("wkq8", CT, 2 * C)
            wv = wload("wv8", CT, C)
            wr16 = wload("wr16", CT, C, BF16)
            w1 = wload("w1_8", CT, DFF, tag="wmlp_a")
            w2 = wload("w2_8", FT, C, tag="wmlp_b")
            ident8 = wp.tile([128, 128], F8, tag="ident8")
            nc.sync.dma_start(out=ident8, in_=io["ident8"][:, :])
            ident16 = wp.tile([128, 128], BF16, tag="ident16")
            nc.sync.dma_start(out=ident16, in_=io["ident16"][:, :])
            ident32 = wp.tile([128, 128], F32, tag="ident32")
            nc.sync.dma_start(out=ident32, in_=io["ident32"][:, :])
            ones8 = wp.tile([128, 1], F8, tag="ones8")
            nc.sync.dma_start(out=ones8, in_=io["ones8"][:, :])

            def dump(name, src_ap):
                if DBG:
                    dt = stg.tile(list(src_ap.shape), F32, tag=f"dbg_{name}",
                                  name=f"dbg_{name}", bufs=1)
                    nc.vector.tensor_copy(dt, src_ap)
                    nc.sync.dma_start(out=io[name].__getitem__(
                        tuple([slice(None)] * len(src_ap.shape))), in_=dt)

            # ---------------- helpers ----------------
            def ln_chunk(src_t, tiles, y, c0, m, tag, div=1.0):
                """Write rsqrt(var+eps)/div for tiles[c0:c0+m] into y[:, c0:c0+m]."""
                mvg = tp.tile([128, m, 2], F32, tag=f"mvg_{tag}",
                              name=f"mvg_{tag}_{c0}", bufs=2)
                for j in range(m):
                    st6 = tp.tile([128, 6], BF16, tag="st6", bufs=4)
                    nc.vector.bn_stats(out=st6, in_=src_t[:, tiles[c0 + j], :])
                    nc.vector.bn_aggr(out=mvg[:, j, :], in_=st6)
                vpe = tp.tile([128, m], F32, tag=f"vpe_{tag}",
                              name=f"vpe_{tag}_{c0}", bufs=2)
                nc.vector.tensor_scalar(out=vpe, in0=mvg[:, :, 1],
                                        scalar1=EPS_LN, scalar2=None,
                                        op0=OP.add)
                ys = y[:, c0:c0 + m]
                nc.vector.tensor_scalar(out=ys.bitcast(I32),
                                        in0=vpe.bitcast(I32),
                                        scalar1=1, scalar2=None,
                                        op0=OP.logical_shift_right)
                nc.vector.tensor_scalar(out=ys.bitcast(I32),
                                        in0=ys.bitcast(I32),
                                        scalar1=-1, scalar2=MAGIC,
                                        op0=OP.mult, op1=OP.add)
                t_ = tp.tile([128, m], F32, tag=f"nt_{tag}",
                             name=f"nt_{tag}_{c0}", bufs=2)
                for it in range(2):
                    last = it == 1
                    nc.gpsimd.tensor_tensor(out=t_, in0=ys, in1=ys, op=OP.mult)
                    nc.gpsimd.tensor_tensor(out=t_, in0=t_, in1=vpe, op=OP.mult)
                    nc.vector.tensor_scalar(
                        out=t_, in0=t_,
                        scalar1=(-0.5 / div) if last else -0.5,
                        scalar2=(1.5 / div) if last else 1.5,
                        op0=OP.mult, op1=OP.add)
                    nc.gpsimd.tensor_tensor(out=ys, in0=ys, in1=t_, op=OP.mult)

            def ln_rs(src_t, tiles, tag, div=1.0, chunk=8):
                n = len(tiles)
                y = stg.tile([128, n], F32, tag=f"rs_{tag}", name=f"rs_{tag}")
                for c0 in range(0, n, chunk):
                    ln_chunk(src_t, tiles, y, c0, min(chunk, n - c0), tag, div)
                return y

            def bscale(sc, j0, n, width=C):
                """[128, n] slice of sc -> stride-0 broadcast [128, n, width]."""
                return sc[:, j0:j0 + n].unsqueeze(-1).broadcast_to((128, n, width))

            def cm_group(src_t, rs, x16, cm, g, tag):
                """tiles 4g..4g+4: mult (alt Pool/DVE) + DMA xbar transposes."""
                with nc.allow_low_precision(reason="bf16 matmul operand"):
                    eng = nc.vector if g % 2 == 0 else nc.gpsimd
                    eng.tensor_tensor(
                        out=x16[:, g * 4:(g + 1) * 4, :],
                        in0=src_t[:, g * 4:(g + 1) * 4, :],
                        in1=bscale(rs, g * 4, 4), op=OP.mult)
                    nc.sync.dma_start_transpose(
                        cm[:, g * 4:(g + 1) * 4, :, :].rearrange(
                            "p a c k -> p (a c) k"),
                        x16[:, g * 4:(g + 1) * 4, :].rearrange("p a b -> p (a b)"))

            def cm_tiles(tag):
                """cm layout: [128, tok-block, ct, 128] so a 4-tile xbar
                transpose writes one contiguous [128, 1024] region."""
                x16 = ap.tile([128, NTH, C], BF16, tag=f"tm_{tag}",
                              name=f"tm_{tag}")
                cm = ap.tile([128, NTH, CT, 128], BF16, tag=f"cm_{tag}",
                             name=f"cm_{tag}")
                return x16, cm

            # ================= EfficientAttention =================
            # ================= EfficientAttention =================
            # ================= EfficientAttention =================
            # sc_kq = rs1/SW for ALL 32 tiles (stats replicated across pair)
            sc_kq = ln_rs(xt, list(range(NT)), "ln1", div=SW)

            kq8 = ap.tile([128, NT, 2 * C], F8, tag="kq8")
            ksums = stg.tile([128, NT], F32, tag="ksums")
            rinv64 = stg.tile([128, NT], F32, tag="rinv64")
            ps_s0 = pacc.tile([128, C], F32, tag="ps_s0")
            ps_s1 = pacc.tile([128, C], F32, tag="ps_s1")

            for p in range(NT // 2):
                for i in range(2):
                    t = 2 * p + i
                    ps = pmm.tile([128, 2 * C], F32, tag="mm")
                    nc.tensor.matmul(ps, xcm[:, :, t * 128:(t + 1) * 128],
                                     wkq, start=True, stop=True, perf_mode=DR)
                    with nc.allow_low_precision(reason="fp8 exp"):
                        nc.scalar.activation(kq8[:, t, :], ps, AF.Exp,
                                             scale=sc_kq[:, t:t + 1])
                # ksum + SK8/ksum for the pair
                nc.vector.tensor_reduce(
                    ksums[:, 2 * p:2 * p + 2].unsqueeze(-1),
                    kq8[:, 2 * p:2 * p + 2, 0:C], axis=AX.X, op=OP.add)
                nc.vector.reciprocal(rinv64[:, 2 * p:2 * p + 2],
                                     ksums[:, 2 * p:2 * p + 2])
                nc.vector.tensor_scalar(out=rinv64[:, 2 * p:2 * p + 2],
                                        in0=rinv64[:, 2 * p:2 * p + 2],
                                        scalar1=SK8, scalar2=None, op0=OP.mult)
                with nc.allow_low_precision(reason="fp8 softmax-k scale"):
                    keng = nc.vector if p % 3 == 0 else nc.gpsimd
                    keng.tensor_tensor(
                        out=kq8[:, 2 * p:2 * p + 2, 0:C],
                        in0=kq8[:, 2 * p:2 * p + 2, 0:C],
                        in1=bscale(rinv64, 2 * p, 2), op=OP.mult)
                # S accumulation (DoubleRow over the token-tile pair)
                st_, sp_ = (p == 0), (p == NT // 2 - 1)
                nc.tensor.matmul(ps_s0, kq8[:, 2 * p:2 * p + 2, C:C + 128],
                                 kq8[:, 2 * p:2 * p + 2, 0:C],
                                 start=st_, stop=sp_, perf_mode=DR)
                nc.tensor.matmul(ps_s1, kq8[:, 2 * p:2 * p + 2, C + 128:2 * C],
                                 kq8[:, 2 * p:2 * p + 2, 0:C],
                                 start=st_, stop=sp_, perf_mode=DR)

            dump("d_rs1", sc_kq)
            dump("d_kq", kq8[:, 0, :])

            # V channel-major for own half (fp8, carries SW)
            vcm = ap.tile([128, CT, T], F8, tag="vcm")
            for ch in range(4):
                for ct in range(CT):
                    ps = pmm.tile([128, 512], F32, tag="mm")
                    nc.tensor.matmul(ps, wv[:, :, ct * 128:(ct + 1) * 128],
                                     xcm[:, :, ch * 512:(ch + 1) * 512],
                                     start=True, stop=True, perf_mode=DR)
                    with nc.allow_low_precision(reason="fp8 V"):
                        nc.scalar.activation(vcm[:, ct, ch * 512:(ch + 1) * 512],
                                             ps, AF.Identity)

            # S_T evac (bf16) + column sums via Act accumulate
            s_t = stg.tile([128, CT, C], BF16, tag="s_t")
            csum = stg.tile([128, CT], F32, tag="csum")
            with nc.allow_low_precision(reason="bf16 S"):
                nc.scalar.activation(s_t[:, 0, :], ps_s0, AF.Identity,
                                     accum_out=csum[:, 0:1])
                nc.scalar.activation(s_t[:, 1, :], ps_s1, AF.Identity,
                                     accum_out=csum[:, 1:2])
            dump("d_st", s_t)
            dump("d_csum", csum)
            cinv = stg.tile([128, CT], F32, tag="cinv")
            nc.vector.reciprocal(cinv, csum)
            wrs = stg.tile([128, CT, C], BF16, tag="wrs")
            with nc.allow_low_precision(reason="bf16 wrs"):
                nc.gpsimd.tensor_tensor(out=wrs, in0=wr16,
                                        in1=bscale(cinv, 0, CT), op=OP.mult)
            # S2[dk, o] = sum_dq S_T[dq, dk] * wrs[dq, o]   (fp8 out, x SWR*SK8)
            s2 = stg.tile([128, CT, C], F8, tag="s2")
            for mt in range(CT):
                ps = pmm.tile([128, C], F32, tag="mm")
                nc.tensor.matmul(ps, s_t[:, 0, mt * 128:(mt + 1) * 128],
                                 wrs[:, 0, :], start=True, stop=False)
                nc.tensor.matmul(ps, s_t[:, 1, mt * 128:(mt + 1) * 128],
                                 wrs[:, 1, :], start=False, stop=True)
                with nc.allow_low_precision(reason="fp8 S2"):
                    nc.scalar.activation(s2[:, mt, :], ps, AF.Identity)

            # att = V @ S2 ; add1 = x + att * rs1 / (SW * SWR * SK8)
            sc_att = stg.tile([128, NTH], F32, tag="sc_att")
            nc.vector.tensor_scalar(out=sc_att, in0=sc_kq[:, 0:NTH],
                                    scalar1=1.0 / SWR,
                                    scalar2=None, op0=OP.mult)
            add1 = ap.tile([128, NTH, C], F32, tag="residA")
            rs2 = stg.tile([128, NTH], F32, tag="rs_ln2")
            x2n16, x2cm = cm_tiles("m")
            for G in range(4):
                for sg in range(2):
                    g = G * 2 + sg
                    ps = pmm.tile([128, 2, C], F32, tag="mm")
                    for j in range(2):
                        t = g * 2 + j
                        nc.tensor.matmul(ps[:, j, :], vcm[:, :, t * 128:(t + 1) * 128],
                                         s2, start=(j == 0), stop=(j == 1),
                                         perf_mode=DR)
                    for j in range(2):
                        t = g * 2 + j
                        nc.vector.scalar_tensor_tensor(
                            out=add1[:, t, :], in0=ps[:, j, :],
                            scalar=sc_att[:, t:t + 1], in1=xt[:, t, :],
                            op0=OP.mult, op1=OP.add)
                ln_chunk(add1, list(range(NTH)), rs2, G * 4, 4, "ln2")
                cm_group(add1, rs2, x2n16, x2cm, G, "m")

            # ================= MLP 1 =================
            def mlp(resid, xcm16, w_a, w_b, out_tile, final_dma, post_group=None):
                """out = resid + W_b.T @ gelu(W_a.T @ xcm16) / SW.
                post_group(G) is emitted after each 4-tile out group."""
                h8 = ap.tile([128, FT, T], F8, tag="h8")
                for ch in range(4):
                    for ft in range(FT):
                        ps = pmm.tile([128, 512], F32, tag="mm")
                        for kt in range(CT):
                            nc.tensor.matmul(
                                ps, w_a[:, kt, ft * 128:(ft + 1) * 128],
                                xcm16[:, ch * 4:(ch + 1) * 4, kt, :],
                                start=(kt == 0), stop=(kt == CT - 1))
                        with nc.allow_low_precision(reason="fp8 h"):
                            nc.scalar.activation(h8[:, ft, ch * 512:(ch + 1) * 512],
                                                 ps, AF.Gelu, scale=1.0 / SW)
                for G in range(4):
                    for sg in range(2):
                        g = G * 2 + sg
                        ps = pmm.tile([128, 2, C], F32, tag="mm")
                        for j in range(2):
                            t = g * 2 + j
                            for fp in range(FT // 2):
                                nc.tensor.matmul(
                                    ps[:, j, :],
                                    h8[:, 2 * fp:2 * fp + 2, t * 128:(t + 1) * 128],
                                    w_b[:, 2 * fp:2 * fp + 2, :],
                                    start=(fp == 0 and j == 0),
                                    stop=(fp == FT // 2 - 1 and j == 1),
                                    perf_mode=DR)
                        if final_dma:
                            ot = tp.tile([128, 2, C], F32, tag="out_sb", bufs=4)
                            nc.vector.scalar_tensor_tensor(
                                out=ot, in0=ps, scalar=1.0 / SW,
                                in1=resid[:, g * 2:g * 2 + 2, :],
                                op0=OP.mult, op1=OP.add)
                            nc.sync.dma_start(
                                out=io["y"][:, :].rearrange(
                                    "(tt p) c -> p tt c", p=128)[:, g * 2:g * 2 + 2, :],
                                in_=ot)
                        else:
                            nc.vector.scalar_tensor_tensor(
                                out=out_tile[:, g * 2:g * 2 + 2, :], in0=ps,
                                scalar=1.0 / SW, in1=resid[:, g * 2:g * 2 + 2, :],
                                op0=OP.mult, op1=OP.add)
                    if post_group is not None:
                        post_group(G)
                return out_tile

            rs2 = ln_rs(add1, list(range(NTH)), "ln2", chunk=4)
            dump("d_rs2", rs2)
            add2 = ap.tile([128, NTH, C], F32, tag="residB")
            rs3 = stg.tile([128, NTH], F32, tag="rs_ln3")
            x3n16, n3cm = cm_tiles("ca")

            def post_mlp1(G):
                ln_chunk(add2, list(range(NTH)), rs3, G * 4, 4, "ln3")
                cm_group(add2, rs3, x3n16, n3cm, G, "ca")

            mlp(add1, x2cm, w1, w2, add2, False, post_group=post_mlp1)

            # ================= ChannelAttention =================
            dump("d_add2", add2[:, 0, :])
            qkw = wload("qk8", CT, 2 * C)
            vvw = wload("vv8", CT, C)
            pjw = wload("pj8", CT, C)
            bdm = wp.tile([128, CT, 128], BF16, tag="bdm")
            nc.sync.dma_start(out=bdm, in_=io["bdmask"][:, :, :])
            temp_sb = wp.tile([128, CT], F32, tag="temp")
            nc.sync.dma_start(out=temp_sb, in_=io["temp_c"][:, :])


            # qk projections (fp8 x SQK); gram + norms accumulated on PE
            qkt8 = ap.tile([128, NTH, 2 * C], F8, tag="qkt8")
            ps_a0 = pacc.tile([128, C], F32, tag="ps_s0")
            ps_a1 = pacc.tile([128, C], F32, tag="ps_s1")
            ps_nrm = pacc.tile([1, 2 * C], F32, tag="ps_nrm")
            sq16 = ap.tile([128, NTH, 2 * C], BF16, tag="sq16")
            for p in range(NTH // 2):
                for i in range(2):
                    t = 2 * p + i
                    ps = pmm.tile([128, 2 * C], F32, tag="mm")
                    for kt in range(CT):
                        nc.tensor.matmul(ps, n3cm[:, t, kt, :],
                                         qkw[:, kt, :], start=(kt == 0),
                                         stop=(kt == CT - 1))
                    with nc.allow_low_precision(reason="fp8 qk"):
                        nc.scalar.activation(qkt8[:, t, :], ps, AF.Identity)
                        nc.vector.tensor_tensor(out=sq16[:, t, :],
                                                in0=qkt8[:, t, :],
                                                in1=qkt8[:, t, :], op=OP.mult)
                st_, sp_ = (p == 0), (p == NTH // 2 - 1)
                nc.tensor.matmul(ps_nrm, ones8, sq16[:, 2 * p, :],
                                 start=st_, stop=False)
                nc.tensor.matmul(ps_nrm, ones8, sq16[:, 2 * p + 1, :],
                                 start=False, stop=sp_)
                nc.tensor.matmul(ps_a0, qkt8[:, 2 * p:2 * p + 2, 0:128],
                                 qkt8[:, 2 * p:2 * p + 2, C:2 * C],
                                 start=st_, stop=sp_, perf_mode=DR)
                nc.tensor.matmul(ps_a1, qkt8[:, 2 * p:2 * p + 2, 128:C],
                                 qkt8[:, 2 * p:2 * p + 2, C:2 * C],
                                 start=st_, stop=sp_, perf_mode=DR)

            dump("d_qkt", qkt8[:, 0, :])

            # pack the used per-head diagonal 32x32 gram blocks + norms
            ca_tx = stg.tile([128, CCW], F32, tag="ca_tx")
            for hh in range(H_CH):
                ct, r0 = hh // 4, (hh % 4) * HD
                src_ps = ps_a0 if ct == 0 else ps_a1
                nc.vector.tensor_copy(ca_tx[r0:r0 + HD, ct * HD:(ct + 1) * HD],
                                      src_ps[r0:r0 + HD, hh * HD:(hh + 1) * HD])
            nrm_sb = stg.tile([1, 2 * C], F32, tag="nrm_sb")
            nc.vector.tensor_copy(nrm_sb, ps_nrm)
            ps_fl = pmm.tile([128, 2 * CT], F32, tag="mm", name="ps_fl")
            for i in range(2 * CT):
                nc.tensor.matmul(ps_fl[:, i:i + 1],
                                 nrm_sb[0:1, i * 128:(i + 1) * 128],
                                 ident32[0:1, 0:1], is_transpose=True,
                                 start=(i == 0), stop=(i == 2 * CT - 1))
            nc.vector.tensor_copy(ca_tx[:, 2 * HD:CCW], ps_fl)
            nc.sync.dma_start(out=cc_in[:, :], in_=ca_tx[:, :])
            nc.gpsimd.collective_compute(
                "AllGather", OP.bypass, replica_groups=REPLICA_GROUPS,
                ins=[cc_in[:, :]], outs=[cc_out[:, :, :]])

            # MLP2 weights arrive during the collective
            w3 = wload("w3_8", CT, DFF, tag="wmlp_a")
            w4 = wload("w4_8", FT, C, tag="wmlp_b")

            # v channel-major (overlaps the collective)
            vcm3 = ap.tile([128, CT, T], F8, tag="vcm")
            for ch in range(4):
                for ct in range(CT):
                    ps = pmm.tile([128, 512], F32, tag="mm")
                    for kt in range(CT):
                        nc.tensor.matmul(ps, vvw[:, kt, ct * 128:(ct + 1) * 128],
                                         n3cm[:, ch * 4:(ch + 1) * 4, kt, :],
                                         start=(kt == 0), stop=(kt == CT - 1))
                    with nc.allow_low_precision(reason="fp8 v"):
                        nc.scalar.activation(vcm3[:, ct, ch * 512:(ch + 1) * 512],
                                             ps, AF.Identity)


            # ---- post-collective epilogue ----
            ca_rx = stg.tile([128, 2, CCW], F32, tag="ca_rx")
            nc.sync.dma_start(out=ca_rx, in_=cc_out[:, :, :].rearrange("r p w -> p r w"))
            ca_tot = stg.tile([128, CCW], F32, tag="ca_tot")
            nc.vector.tensor_tensor(out=ca_tot, in0=ca_rx[:, 0, :],
                                    in1=ca_rx[:, 1, :], op=OP.add)

            dump("d_catot", ca_tot)
            nktot = ca_tot[:, 2 * HD:CCW]    # [128, 4]: qsumsq-cols | ksumsq-cols
            # inv norms via Newton rsqrt (columns, f32)
            invn = stg.tile([128, 2 * CT], F32, tag="invn")
            nw_t = tp.tile([128, 2 * CT], F32, tag="nw_t")
            nc.vector.tensor_scalar(out=invn.bitcast(I32), in0=nktot.bitcast(I32),
                                    scalar1=1, scalar2=None,
                                    op0=OP.logical_shift_right)
            nc.vector.tensor_scalar(out=invn.bitcast(I32), in0=invn.bitcast(I32),
                                    scalar1=-1, scalar2=MAGIC,
                                    op0=OP.mult, op1=OP.add)
            for _ in range(2):
                nc.vector.tensor_tensor(out=nw_t, in0=invn, in1=invn, op=OP.mult)
                nc.vector.tensor_tensor(out=nw_t, in0=nw_t, in1=nktot, op=OP.mult)
                nc.vector.tensor_scalar(out=nw_t, in0=nw_t, scalar1=-0.5,
                                        scalar2=1.5, op0=OP.mult, op1=OP.add)
                nc.vector.tensor_tensor(out=invn, in0=invn, in1=nw_t, op=OP.mult)
            # scale invq by temperature and the poly-exp 1/4 folding
            invq = stg.tile([128, CT], F32, tag="invq")
            nc.vector.tensor_tensor(out=invq, in0=invn[:, 0:CT], in1=temp_sb,
                                    op=OP.mult)
            nc.vector.tensor_scalar(out=invq, in0=invq, scalar1=0.25,
                                    scalar2=None, op0=OP.mult)
            # invk back to a row [1, C] via PE transpose, broadcast to [128, C]
            ps_kf = pmm.tile([1, C], F32, tag="mm", name="ps_kf")
            for ct in range(CT):
                nc.tensor.matmul(ps_kf[0:1, ct * 128:(ct + 1) * 128],
                                 invn[:, CT + ct:CT + ct + 1],
                                 ident32, is_transpose=True,
                                 start=(ct == 0), stop=(ct == 1))
            invk_row = tp.tile([1, C], F32, tag="invk_row")
            nc.vector.tensor_copy(invk_row, ps_kf)
            ones_row16 = tp.tile([1, 128], BF16, tag="ones_row16")
            nc.vector.memset(ones_row16, 1.0)
            invk_row16 = tp.tile([1, C], BF16, tag="invk_row16")
            with nc.allow_low_precision(reason="bf16 bcast operand"):
                nc.vector.tensor_copy(invk_row16, invk_row)
            ps_bk = pmm.tile([128, C], F32, tag="mm", name="ps_bk")
            nc.tensor.matmul(ps_bk, ones_row16, invk_row16, start=True, stop=True)
            bk = stg.tile([128, C], F32, tag="bk")
            nc.vector.tensor_copy(bk, ps_bk)

            # logits = gram * invq(part) * invk(elem); per-head blocks only
            attn_l = stg.tile([128, CT, 128], F32, tag="attn_l")
            nc.vector.memset(attn_l, 0.0)
            for hh in range(H_CH):
                ct, r0 = hh // 4, (hh % 4) * HD
                nc.vector.scalar_tensor_tensor(
                    out=attn_l[r0:r0 + HD, ct, r0:r0 + HD],
                    in0=ca_tot[r0:r0 + HD, ct * HD:(ct + 1) * HD],
                    scalar=invq[r0:r0 + HD, ct:ct + 1],
                    in1=bk[r0:r0 + HD, hh * HD:(hh + 1) * HD],
                    op0=OP.mult, op1=OP.mult)

            # exp via (1 + u + u^2/2 + u^3/6)^4; u = logits/4 folded into invq
            u = attn_l
            pe_ = stg.tile([128, CT, 128], F32, tag="attn_p")
            nc.vector.tensor_scalar(out=pe_, in0=u, scalar1=1.0 / 6.0,
                                    scalar2=0.5, op0=OP.mult, op1=OP.add)
            nc.vector.tensor_tensor(out=pe_, in0=pe_, in1=u, op=OP.mult)
            nc.vector.tensor_scalar(out=pe_, in0=pe_, scalar1=1.0, scalar2=None,
                                    op0=OP.add)
            nc.vector.tensor_tensor(out=pe_, in0=pe_, in1=u, op=OP.mult)
            nc.vector.tensor_scalar(out=pe_, in0=pe_, scalar1=1.0, scalar2=None,
                                    op0=OP.add)
            nc.vector.tensor_tensor(out=pe_, in0=pe_, in1=pe_, op=OP.mult)
            nc.vector.tensor_tensor(out=pe_, in0=pe_, in1=pe_, op=OP.mult)
            # mask off-block entries, row-normalize
            with nc.allow_low_precision(reason="bf16 mask"):
                nc.vector.tensor_tensor(out=pe_, in0=pe_, in1=bdm, op=OP.mult)
            rsum = tp.tile([128, CT], F32, tag="attn_rs")
            nc.vector.tensor_reduce(rsum.unsqueeze(-1), pe_, axis=AX.X, op=OP.add)
            rinv = tp.tile([128, CT], F32, tag="attn_ri")
            nc.vector.reciprocal(rinv, rsum)
            attn16 = stg.tile([128, CT, 128], BF16, tag="attn16")
            with nc.allow_low_precision(reason="bf16 attn"):
                nc.vector.tensor_tensor(out=attn16, in0=pe_,
                                        in1=bscale(rinv, 0, CT, 128), op=OP.mult)

            dump("d_attn", attn16)

            # transpose attn blocks -> lhsT; attn @ v ; proj ; add3
            at_bd = stg.tile([128, CT, 128], BF16, tag="at_bd")
            for ct in range(CT):
                ps_at = pmmt.tile([128, 128], BF16, tag="mmt", name=f"ps_at{ct}")
                nc.tensor.transpose(ps_at, attn16[:, ct, :], ident16)
                with nc.allow_low_precision(reason="bf16 attn lhsT"):
                    nc.vector.tensor_copy(at_bd[:, ct, :], ps_at)

            cac = ap.tile([128, CT, T], F8, tag="cac")
            for ct in range(CT):
                for ch in range(4):
                    ps = pmm.tile([128, 512], F32, tag="mm")
                    nc.tensor.matmul(ps, at_bd[:, ct, :],
                                     vcm3[:, ct, ch * 512:(ch + 1) * 512],
                                     start=True, stop=True)
                    with nc.allow_low_precision(reason="fp8 cac"):
                        nc.scalar.activation(cac[:, ct, ch * 512:(ch + 1) * 512],
                                             ps, AF.Identity)

            add3 = ap.tile([128, NTH, C], F32, tag="residA")
            rs4 = stg.tile([128, NTH], F32, tag="rs_ln4")
            x4n16, x4cm = cm_tiles("f")
            for G in range(4):
                for sg in range(2):
                    g = G * 2 + sg
                    ps = pmm.tile([128, 2, C], F32, tag="mm")
                    for j in range(2):
                        t = g * 2 + j
                        nc.tensor.matmul(ps[:, j, :], cac[:, :, t * 128:(t + 1) * 128],
                                         pjw, start=(j == 0), stop=(j == 1),
                                         perf_mode=DR)
                    nc.vector.scalar_tensor_tensor(
                        out=add3[:, g * 2:g * 2 + 2, :], in0=ps,
                        scalar=1.0 / (SW * SW), in1=add2[:, g * 2:g * 2 + 2, :],
                        op0=OP.mult, op1=OP.add)
                ln_chunk(add3, list(range(NTH)), rs4, G * 4, 4, "ln4")
                cm_group(add3, rs4, x4n16, x4cm, G, "f")

            # ================= MLP 2 (writes y) =================
            mlp(add3, x4cm, w3, w4, None, True)

    nc.compile()
    _CACHE["nc"] = nc
    return nc


def prep_host(inputs):
    """Weight/layout prep shared by all cores (no arithmetic on x)."""
    f8 = ml_dtypes.float8_e4m3
    f = lambda k: np.asarray(inputs[k], np.float32)
    for k in ("ln1_b", "ln2_b", "ln3_b", "ln4_b", "m1_b1", "m1_b2",
              "m2_b1", "m2_b2", "proj_b"):
        assert np.abs(f(k)).max() == 0.0, f"{k} nonzero; bias path not emitted"
    g1, g2, g3, g4 = f("ln1_g"), f("ln2_g"), f("ln3_g"), f("ln4_g")

    def fold(w, g, center=True):
        """M = Pc @ (diag(g) @ w.T): [in, out] with LN gamma + centering."""
        m = (w * g[None, :]).T.astype(np.float32)
        if center:
            m = m - m.mean(axis=0, keepdims=True)
        return m

    def arr_ct(m, scale):
        """[C_in, O] -> [128, C_in//128, O] fp8 with scale."""
        ci, o = m.shape
        return np.ascontiguousarray(
            (m * scale).reshape(ci // 128, 128, o).transpose(1, 0, 2)
        ).astype(f8)

    qkv_w = f("qkv_w")
    wkq = np.concatenate([fold(f("wk"), g1), fold(f("wq"), g1)], axis=1)
    qk_ca = fold(qkv_w[:2 * C], g3)          # [C, 2C]
    vv_ca = fold(qkv_w[2 * C:], g3)          # [C, C]
    w1 = fold(f("m1_w1"), g2)
    w3 = fold(f("m2_w1"), g4)
    w2 = f("m1_w2").T                        # [DFF, C]
    w4 = f("m2_w2").T
    wr = f("wr").T                           # [C, C]
    pj = f("proj_w").T

    bdmask = np.zeros((128, CT, 128), np.float32)
    for hh in range(H_CH):
        ct, r0 = hh // 4, (hh % 4) * HD
        bdmask[r0:r0 + HD, ct, r0:r0 + HD] = 1.0

    temp = np.repeat(f("temperature").reshape(H_CH), HD).reshape(CT, 128).T

    return {
        "wkq8": arr_ct(wkq, SW),
        "wv8": arr_ct(fold(f("wv"), g1), SW),
        "wr16": np.ascontiguousarray(
            (wr * SWR).reshape(CT, 128, C).transpose(1, 0, 2)
        ).astype(ml_dtypes.bfloat16),
        "qk8": arr_ct(qk_ca, SQK),
        "vv8": arr_ct(vv_ca, SW),
        "pj8": arr_ct(pj, SW),
        "w1_8": arr_ct(w1, SW),
        "w2_8": arr_ct(w2, SW),
        "w3_8": arr_ct(w3, SW),
        "w4_8": arr_ct(w4, SW),
        "ident8": np.eye(128, dtype=f8),
        "ident16": np.eye(128, dtype=ml_dtypes.bfloat16),
        "ones8": np.ones((128, 1), f8),
        "bdmask": bdmask.astype(ml_dtypes.bfloat16),
        "temp_c": np.ascontiguousarray(temp, np.float32),
        "ident32": np.eye(128, dtype=np.float32),
    }


def make_in_maps(inputs):
    shared = prep_host(inputs)
    f8 = ml_dtypes.float8_e4m3
    x = np.asarray(inputs["x"], np.float32)
    in_maps = []
    for c in range(NCORES):
        b, hlf = c // 2, c % 2
        # own half first, peer half second (layout only, no arithmetic)
        xo = np.concatenate([x[b, hlf * T:(hlf + 1) * T, :],
                             x[b, (1 - hlf) * T:(2 - hlf) * T, :]], axis=0)
        m = dict(shared)
        m["xt16"] = np.ascontiguousarray(
            xo.reshape(NT, 128, C).transpose(1, 0, 2)).astype(ml_dtypes.bfloat16)
        m["xcm8"] = np.ascontiguousarray(
            xo.T.reshape(CT, 128, N).transpose(1, 0, 2)).astype(f8)
        in_maps.append(m)
    return in_maps


def assemble(results):
    y = np.empty((B, N, C), np.float32)
    for c in range(NCORES):
        b, hlf = c // 2, c % 2
        y[b, hlf * T:(hlf + 1) * T, :] = results[c]["y"]
    return y


def kernel(**inputs):
    from concourse.bass_utils import run_bass_kernel_spmd

    nc = build_program()
    in_maps = make_in_maps(inputs)
    res = run_bass_kernel_spmd(nc, in_maps, list(range(NCORES)))
    return assemble(res.results)


# revision 29
# speedup vs baseline: 1.1098x; 1.0645x over previous
"""DualTransformerBlock Trainium2 kernel (v2).

Strategy (8 cores: core c -> sample b=c//2, token half h=c%2, T=2048):
  - EfficientAttention reassociated: att = V @ ((K^T Q)/colsum @ wr.T); the
    [N,N] context never materializes.  The K/Q-side stats (exp projections and
    the [C,C] S matrix) are computed REDUNDANTLY for all 4096 tokens on both
    cores of a pair, so no collective is needed for EA.
  - Each core's x tensors are ordered with ITS half first (tiles 0..15 = own
    half, 16..31 = peer half); the S/stat sums are order-invariant, so one
    SPMD program serves all cores.
  - ChannelAttention still needs one cross-half reduction (per-head gram +
    q/k norms); exchanged via AllGather (cheaper than AllReduce in the cost
    model) and summed locally.
  - LayerNorm: mean-centering is folded into the following weight matrices on
    the host ((x-mean) @ W == x @ (Pc W)); the per-token 1/sigma comes from a
    bit-trick Newton rsqrt (no activation-table use) and is fused into Act
    scale= or evacuation scalars.
  - Heavy matmuls run in fp8e4m3 with DoubleRow perf mode (2 contraction
    tiles per instruction, 0.5 cyc/row).  Host-side exponent scaling keeps
    fp8 operands out of the subnormal range; scales cancel or fold into
    per-token evacuation scalars.
  - Activation tables: only Exp (EA) and Gelu (MLPs) -> 2 table loads.  The
    tiny per-head channel-attn softmax uses a polynomial exp on DVE/Pool.
"""

import os
import sys

sys.path.insert(0, "/opt/trn_rl_repo")

import numpy as np
import ml_dtypes

import concourse.bass as bass
import concourse.mybir as mybir
from concourse import bacc
from concourse.tile import TileContext

F32 = mybir.dt.float32
BF16 = mybir.dt.bfloat16
F8 = mybir.dt.float8e4
I32 = mybir.dt.int32
AF = mybir.ActivationFunctionType
OP = mybir.AluOpType
AX = mybir.AxisListType
DR = mybir.MatmulPerfMode.DoubleRow

B, N, C = 4, 4096, 256
H_CH = 8
HD = C // H_CH          # 32
DFF = 4 * C             # 1024
EPS_LN = 1e-5

NCORES = 8
T = N // 2              # 2048 tokens per core half
NT = N // 128           # 32 token tiles (full sample)
NTH = T // 128          # 16 token tiles (own half)
CT = C // 128           # 2 channel tiles
FT = DFF // 128         # 8 ff tiles
REPLICA_GROUPS = [[0, 1], [2, 3], [4, 5], [6, 7]]
CCW = 2 * HD + 2 * CT   # collective payload width (68)

# fp8 exponent scales (host folds these into weights; device descales)
SW = 32.0       # generic weight scale (wkq, wv, w1..w4, qkv-v, proj)
SQK = 8.0       # channel-attn qk scale
SK8 = 64.0      # normalized-k softmax scale
SWR = 256.0     # wr scale
MAGIC = 0x5F3759DF

_CACHE = {}


def build_program():
    if "nc" in _CACHE:
        return _CACHE["nc"]
    nc = bacc.Bacc(None, target_bir_lowering=False)

    io = {}

    def param(name, shape, dt=F32):
        io[name] = nc.declare_dram_parameter(name, list(shape), dt, isOutput=False)

    param("xt16", (128, NT, C), BF16)       # token-major x, all tokens, own half first
    param("xcm8", (128, CT, N), F8)         # channel-major x, all tokens
    param("wkq8", (128, CT, 2 * C), F8)     # [K|Q] proj  (centered, g1, xSW)
    param("wv8", (128, CT, C), F8)          # EA values proj (centered, g1, xSW)
    param("wr16", (128, CT, C), BF16)       # EA out proj (xSWR)
    param("qk8", (128, CT, 2 * C), F8)      # CA [q|k] proj (centered, g3, xSQK)
    param("vv8", (128, CT, C), F8)          # CA v proj (centered, g3, xSW)
    param("pj8", (128, CT, C), F8)          # CA out proj (xSW)
    param("w1_8", (128, CT, DFF), F8)       # MLP1 fc1 (centered, g2, xSW)
    param("w2_8", (128, FT, C), F8)         # MLP1 fc2 (xSW)
    param("w3_8", (128, CT, DFF), F8)       # MLP2 fc1 (centered, g4, xSW)
    param("w4_8", (128, FT, C), F8)         # MLP2 fc2 (xSW)
    param("ident8", (128, 128), F8)
    param("ident16", (128, 128), BF16)
    param("ident32", (128, 128), F32)
    param("ones8", (128, 1), F8)
    param("bdmask", (128, CT, 128), BF16)   # per-head 32x32 block-diag mask
    param("temp_c", (128, CT), F32)         # CA temperature, column layout
    io["y"] = nc.declare_dram_parameter("y", [T, C], F32, isOutput=True)
    DBG = os.environ.get("KDBG", "0") == "1"
    dbg_specs = {
        "d_rs1": (128, NT), "d_kq": (128, 2 * C), "d_st": (128, CT, C),
        "d_csum": (128, CT), "d_s2": (128, CT, C), "d_add1": (128, C),
        "d_vcm": (128, CT, 512), "d_add2": (128, C), "d_qkt": (128, 2 * C),
        "d_catot": (128, CCW), "d_attn": (128, CT, 128), "d_add3": (128, C),
        "d_rs2": (128, NTH),
    }
    if DBG:
        for k, shp in dbg_specs.items():
            io[k] = nc.declare_dram_parameter(k, list(shp), F32, isOutput=True)

    cc_in = nc.dram_tensor("cc_in", [128, CCW], F32)
    cc_out = nc.dram_tensor("cc_out", [2, 128, CCW], F32)

    with TileContext(nc) as tc:
        with (
            tc.tile_pool(name="wpool", bufs=1) as wp,
            tc.tile_pool(name="apool", bufs=1) as ap,
            tc.tile_pool(name="tmp", bufs=3) as tp,
            tc.tile_pool(name="stage", bufs=1) as stg,
            tc.tile_pool(name="pacc", bufs=1, space="PSUM") as pacc,
            tc.tile_pool(name="pmm", bufs=3, space="PSUM") as pmm,
            tc.tile_pool(name="pmmt", bufs=2, space="PSUM") as pmmt,
        ):
            # preload the Exp activation table while DMAs run
            dumm = wp.tile([128, 1], F32, tag="dumm")
            nc.vector.memset(dumm, 0.0)
            nc.scalar.activation(dumm, dumm, AF.Exp)

            # ---------------- input DMA ----------------
            def wload(name, d1, d2, dt=F8, tag=None):
                t = wp.tile([128, d1, d2], dt, tag=tag or name)
                nc.sync.dma_start(out=t, in_=io[name][:, :, :])
                return t

            wkq = wload("wkq8", CT, 2 * C)
            xt = ap.tile([128, NT, C], BF16, tag="xt")
            xcm = ap.tile([128, CT, N], F8, tag="xcm")
            for g in range(4):
                nc.sync.dma_start(
                    out=xcm[:, :, g * (N // 4):(g + 1) * (N // 4)],
                    in_=io["xcm8"][:, :, g * (N // 4):(g + 1) * (N // 4)])
                nc.sync.dma_start(out=xt[:, g * 8:(g + 1) * 8, :],
                                  in_=io["xt16"][:, g * 8:(g + 1) * 8, :])

            wv = wload("wv8", CT, C)
            wr16 = wload("wr16", CT, C, BF16)
            w1 = wload("w1_8", CT, DFF, tag="wmlp_a")
            w2 = wload("w2_8", FT, C, tag="wmlp_b")
            ident8 = wp.tile([128, 128], F8, tag="ident8")
            nc.sync.dma_start(out=ident8, in_=io["ident8"][:, :])
            ident16 = wp.tile([128, 128], BF16, tag="ident16")
            nc.sync.dma_start(out=ident16, in_=io["ident16"][:, :])
            ident32 = wp.tile([128, 128], F32, tag="ident32")
            nc.sync.dma_start(out=ident32, in_=io["ident32"][:, :])
            ones8 = wp.tile([128, 1], F8, tag="ones8")
            nc.sync.dma_start(out=ones8, in_=io["ones8"][:, :])

            def dump(name, src_ap):
                if DBG:
                    dt = stg.tile(list(src_ap.shape), F32, tag=f"dbg_{name}",
                                  name=f"dbg_{name}", bufs=1)
                    nc.vector.tensor_copy(dt, src_ap)
                    nc.sync.dma_start(out=io[name].__getitem__(
                        tuple([slice(None)] * len(src_ap.shape))), in_=dt)

            # ---------------- helpers ----------------
            def ln_chunk(src_t, tiles, y, c0, m, tag, div=1.0):
                """Write rsqrt(var+eps)/div for tiles[c0:c0+m] into y[:, c0:c0+m]."""
                mvg = tp.tile([128, m, 2], F32, tag=f"mvg_{tag}",
                              name=f"mvg_{tag}_{c0}", bufs=2)
                for j in range(m):
                    st6 = tp.tile([128, 6], BF16, tag="st6", bufs=4)
                    nc.vector.bn_stats(out=st6, in_=src_t[:, tiles[c0 + j], :])
                    nc.vector.bn_aggr(out=mvg[:, j, :], in_=st6)
                vpe = tp.tile([128, m], F32, tag=f"vpe_{tag}",
                              name=f"vpe_{tag}_{c0}", bufs=2)
                nc.vector.tensor_scalar(out=vpe, in0=mvg[:, :, 1],
                                        scalar1=EPS_LN, scalar2=None,
                                        op0=OP.add)
                ys = y[:, c0:c0 + m]
                nc.vector.tensor_scalar(out=ys.bitcast(I32),
                                        in0=vpe.bitcast(I32),
                                        scalar1=1, scalar2=None,
                                        op0=OP.logical_shift_right)
                nc.vector.tensor_scalar(out=ys.bitcast(I32),
                                        in0=ys.bitcast(I32),
                                        scalar1=-1, scalar2=MAGIC,
                                        op0=OP.mult, op1=OP.add)
                t_ = tp.tile([128, m], F32, tag=f"nt_{tag}",
                             name=f"nt_{tag}_{c0}", bufs=2)
                for it in range(2):
                    last = it == 1
                    nc.gpsimd.tensor_tensor(out=t_, in0=ys, in1=ys, op=OP.mult)
                    nc.gpsimd.tensor_tensor(out=t_, in0=t_, in1=vpe, op=OP.mult)
                    nc.vector.tensor_scalar(
                        out=t_, in0=t_,
                        scalar1=(-0.5 / div) if last else -0.5,
                        scalar2=(1.5 / div) if last else 1.5,
                        op0=OP.mult, op1=OP.add)
                    nc.gpsimd.tensor_tensor(out=ys, in0=ys, in1=t_, op=OP.mult)

            def ln_rs(src_t, tiles, tag, div=1.0, chunk=8):
                n = len(tiles)
                y = stg.tile([128, n], F32, tag=f"rs_{tag}", name=f"rs_{tag}")
                for c0 in range(0, n, chunk):
                    ln_chunk(src_t, tiles, y, c0, min(chunk, n - c0), tag, div)
                return y

            def bscale(sc, j0, n, width=C):
                """[128, n] slice of sc -> stride-0 broadcast [128, n, width]."""
                return sc[:, j0:j0 + n].unsqueeze(-1).broadcast_to((128, n, width))

            def cm_group(src_t, rs, x16, cm, g, tag):
                """tiles 4g..4g+4: mult (alt Pool/DVE) + DMA xbar transposes."""
                with nc.allow_low_precision(reason="bf16 matmul operand"):
                    eng = nc.vector if g % 2 == 0 else nc.gpsimd
                    eng.tensor_tensor(
                        out=x16[:, g * 4:(g + 1) * 4, :],
                        in0=src_t[:, g * 4:(g + 1) * 4, :],
                        in1=bscale(rs, g * 4, 4), op=OP.mult)
                    nc.sync.dma_start_transpose(
                        cm[:, g * 4:(g + 1) * 4, :, :].rearrange(
                            "p a c k -> p (a c) k"),
                        x16[:, g * 4:(g + 1) * 4, :].rearrange("p a b -> p (a b)"))

            def cm_tiles(tag):
                """cm layout: [128, tok-block, ct, 128] so a 4-tile xbar
                transpose writes one contiguous [128, 1024] region."""
                x16 = ap.tile([128, NTH, C], BF16, tag=f"tm_{tag}",
                              name=f"tm_{tag}")
                cm = ap.tile([128, NTH, CT, 128], BF16, tag=f"cm_{tag}",
                             name=f"cm_{tag}")
                return x16, cm

            # ================= EfficientAttention =================
            # ================= EfficientAttention =================
            # ================= EfficientAttention =================
            # sc_kq = rs1/SW for ALL 32 tiles (stats replicated across pair)
            sc_kq = ln_rs(xt, list(range(NT)), "ln1", div=SW)

            kq8 = ap.tile([128, NT, 2 * C], F8, tag="kq8")
            ksums = stg.tile([128, NT], F32, tag="ksums")
            rinv64 = stg.tile([128, NT], F32, tag="rinv64")
            ps_s0 = pacc.tile([128, C], F32, tag="ps_s0")
            ps_s1 = pacc.tile([128, C], F32, tag="ps_s1")

            for p in range(NT // 2):
                for i in range(2):
                    t = 2 * p + i
                    ps = pmm.tile([128, 2 * C], F32, tag="mm")
                    nc.tensor.matmul(ps, xcm[:, :, t * 128:(t + 1) * 128],
                                     wkq, start=True, stop=True, perf_mode=DR)
                    with nc.allow_low_precision(reason="fp8 exp"):
                        nc.scalar.activation(kq8[:, t, :], ps, AF.Exp,
                                             scale=sc_kq[:, t:t + 1])
                # ksum + SK8/ksum for the pair
                nc.vector.tensor_reduce(
                    ksums[:, 2 * p:2 * p + 2].unsqueeze(-1),
                    kq8[:, 2 * p:2 * p + 2, 0:C], axis=AX.X, op=OP.add)
                nc.vector.reciprocal(rinv64[:, 2 * p:2 * p + 2],
                                     ksums[:, 2 * p:2 * p + 2])
                nc.vector.tensor_scalar(out=rinv64[:, 2 * p:2 * p + 2],
                                        in0=rinv64[:, 2 * p:2 * p + 2],
                                        scalar1=SK8, scalar2=None, op0=OP.mult)
                with nc.allow_low_precision(reason="fp8 softmax-k scale"):
                    keng = nc.vector if p % 3 == 0 else nc.gpsimd
                    keng.tensor_tensor(
                        out=kq8[:, 2 * p:2 * p + 2, 0:C],
                        in0=kq8[:, 2 * p:2 * p + 2, 0:C],
                        in1=bscale(rinv64, 2 * p, 2), op=OP.mult)
                # S accumulation (DoubleRow over the token-tile pair)
                st_, sp_ = (p == 0), (p == NT // 2 - 1)
                nc.tensor.matmul(ps_s0, kq8[:, 2 * p:2 * p + 2, C:C + 128],
                                 kq8[:, 2 * p:2 * p + 2, 0:C],
                                 start=st_, stop=sp_, perf_mode=DR)
                nc.tensor.matmul(ps_s1, kq8[:, 2 * p:2 * p + 2, C + 128:2 * C],
                                 kq8[:, 2 * p:2 * p + 2, 0:C],
                                 start=st_, stop=sp_, perf_mode=DR)

            dump("d_rs1", sc_kq)
            dump("d_kq", kq8[:, 0, :])

            # V channel-major for own half (fp8, carries SW)
            vcm = ap.tile([128, CT, T], F8, tag="vcm")
            for ch in range(4):
                for ct in range(CT):
                    ps = pmm.tile([128, 512], F32, tag="mm")
                    nc.tensor.matmul(ps, wv[:, :, ct * 128:(ct + 1) * 128],
                                     xcm[:, :, ch * 512:(ch + 1) * 512],
                                     start=True, stop=True, perf_mode=DR)
                    with nc.allow_low_precision(reason="fp8 V"):
                        nc.scalar.activation(vcm[:, ct, ch * 512:(ch + 1) * 512],
                                             ps, AF.Identity)

            # S_T evac (bf16) + column sums via Act accumulate
            s_t = stg.tile([128, CT, C], BF16, tag="s_t")
            csum = stg.tile([128, CT], F32, tag="csum")
            with nc.allow_low_precision(reason="bf16 S"):
                nc.scalar.activation(s_t[:, 0, :], ps_s0, AF.Identity,
                                     accum_out=csum[:, 0:1])
                nc.scalar.activation(s_t[:, 1, :], ps_s1, AF.Identity,
                                     accum_out=csum[:, 1:2])
            dump("d_st", s_t)
            dump("d_csum", csum)
            cinv = stg.tile([128, CT], F32, tag="cinv")
            nc.vector.reciprocal(cinv, csum)
            wrs = stg.tile([128, CT, C], BF16, tag="wrs")
            with nc.allow_low_precision(reason="bf16 wrs"):
                nc.gpsimd.tensor_tensor(out=wrs, in0=wr16,
                                        in1=bscale(cinv, 0, CT), op=OP.mult)
            # S2[dk, o] = sum_dq S_T[dq, dk] * wrs[dq, o]   (fp8 out, x SWR*SK8)
            s2 = stg.tile([128, CT, C], F8, tag="s2")
            for mt in range(CT):
                ps = pmm.tile([128, C], F32, tag="mm")
                nc.tensor.matmul(ps, s_t[:, 0, mt * 128:(mt + 1) * 128],
                                 wrs[:, 0, :], start=True, stop=False)
                nc.tensor.matmul(ps, s_t[:, 1, mt * 128:(mt + 1) * 128],
                                 wrs[:, 1, :], start=False, stop=True)
                with nc.allow_low_precision(reason="fp8 S2"):
                    nc.scalar.activation(s2[:, mt, :], ps, AF.Identity)

            # att = V @ S2 ; add1 = x + att * rs1 / (SW * SWR * SK8)
            sc_att = stg.tile([128, NTH], F32, tag="sc_att")
            nc.vector.tensor_scalar(out=sc_att, in0=sc_kq[:, 0:NTH],
                                    scalar1=1.0 / SWR,
                                    scalar2=None, op0=OP.mult)
            add1 = ap.tile([128, NTH, C], F32, tag="residA")
            rs2 = stg.tile([128, NTH], F32, tag="rs_ln2")
            x2n16, x2cm = cm_tiles("m")
            for G in range(4):
                for sg in range(2):
                    g = G * 2 + sg
                    ps = pmm.tile([128, 2, C], F32, tag="mm")
                    for j in range(2):
                        t = g * 2 + j
                        nc.tensor.matmul(ps[:, j, :], vcm[:, :, t * 128:(t + 1) * 128],
                                         s2, start=(j == 0), stop=(j == 1),
                                         perf_mode=DR)
                    for j in range(2):
                        t = g * 2 + j
                        nc.vector.scalar_tensor_tensor(
                            out=add1[:, t, :], in0=ps[:, j, :],
                            scalar=sc_att[:, t:t + 1], in1=xt[:, t, :],
                            op0=OP.mult, op1=OP.add)
                ln_chunk(add1, list(range(NTH)), rs2, G * 4, 4, "ln2")
                cm_group(add1, rs2, x2n16, x2cm, G, "m")

            # ================= MLP 1 =================
            def mlp(resid, xcm16, w_a, w_b, out_tile, final_dma, post_group=None):
                """out = resid + W_b.T @ gelu(W_a.T @ xcm16) / SW, fully
                chunk-interleaved: h for token chunk ch, then the out-proj,
                residual add, and post_group(ch) before the next chunk."""
                for ch in range(4):
                    h8 = tp.tile([128, FT, 512], F8, tag="h8", bufs=2,
                                 name=f"h8_{ch}_{1 if final_dma else 0}")
                    for ft in range(FT):
                        ps = pmm.tile([128, 512], F32, tag="mm")
                        for kt in range(CT):
                            nc.tensor.matmul(
                                ps, w_a[:, kt, ft * 128:(ft + 1) * 128],
                                xcm16[:, ch * 4:(ch + 1) * 4, kt, :],
                                start=(kt == 0), stop=(kt == CT - 1))
                        with nc.allow_low_precision(reason="fp8 h"):
                            nc.scalar.activation(h8[:, ft, :], ps, AF.Gelu,
                                                 scale=1.0 / SW)
                    for sg in range(2):
                        g = ch * 2 + sg
                        ps = pmm.tile([128, 2, C], F32, tag="mm")
                        for j in range(2):
                            tl = sg * 2 + j
                            for fp in range(FT // 2):
                                nc.tensor.matmul(
                                    ps[:, j, :],
                                    h8[:, 2 * fp:2 * fp + 2, tl * 128:(tl + 1) * 128],
                                    w_b[:, 2 * fp:2 * fp + 2, :],
                                    start=(fp == 0 and j == 0),
                                    stop=(fp == FT // 2 - 1 and j == 1),
                                    perf_mode=DR)
                        if final_dma:
                            ot = tp.tile([128, 2, C], F32, tag="out_sb", bufs=4)
                            nc.vector.scalar_tensor_tensor(
                                out=ot, in0=ps, scalar=1.0 / SW,
                                in1=resid[:, g * 2:g * 2 + 2, :],
                                op0=OP.mult, op1=OP.add)
                            nc.sync.dma_start(
                                out=io["y"][:, :].rearrange(
                                    "(tt p) c -> p tt c", p=128)[:, g * 2:g * 2 + 2, :],
                                in_=ot)
                        else:
                            nc.vector.scalar_tensor_tensor(
                                out=out_tile[:, g * 2:g * 2 + 2, :], in0=ps,
                                scalar=1.0 / SW, in1=resid[:, g * 2:g * 2 + 2, :],
                                op0=OP.mult, op1=OP.add)
                    if post_group is not None:
                        post_group(ch)
                return out_tile

            add2 = ap.tile([128, NTH, C], F32, tag="residB")
            rs3 = stg.tile([128, NTH], F32, tag="rs_ln3")
            x3n16, n3cm = cm_tiles("ca")

            def post_mlp1(G):
                ln_chunk(add2, list(range(NTH)), rs3, G * 4, 4, "ln3")
                cm_group(add2, rs3, x3n16, n3cm, G, "ca")

            mlp(add1, x2cm, w1, w2, add2, False, post_group=post_mlp1)

            # ================= ChannelAttention =================
            dump("d_add2", add2[:, 0, :])
            qkw = wload("qk8", CT, 2 * C)
            vvw = wload("vv8", CT, C)
            pjw = wload("pj8", CT, C)
            bdm = wp.tile([128, CT, 128], BF16, tag="bdm")
            nc.sync.dma_start(out=bdm, in_=io["bdmask"][:, :, :])
            temp_sb = wp.tile([128, CT], F32, tag="temp")
            nc.sync.dma_start(out=temp_sb, in_=io["temp_c"][:, :])


            # qk projections (fp8 x SQK); gram + norms accumulated on PE
            qkt8 = ap.tile([128, NTH, 2 * C], F8, tag="qkt8")
            ps_a0 = pacc.tile([128, C], F32, tag="ps_s0")
            ps_a1 = pacc.tile([128, C], F32, tag="ps_s1")
            ps_nrm = pacc.tile([1, 2 * C], F32, tag="ps_nrm")
            sq16 = ap.tile([128, NTH, 2 * C], BF16, tag="sq16")
            for p in range(NTH // 2):
                for i in range(2):
                    t = 2 * p + i
                    ps = pmm.tile([128, 2 * C], F32, tag="mm")
                    for kt in range(CT):
                        nc.tensor.matmul(ps, n3cm[:, t, kt, :],
                                         qkw[:, kt, :], start=(kt == 0),
                                         stop=(kt == CT - 1))
                    with nc.allow_low_precision(reason="fp8 qk"):
                        nc.scalar.activation(qkt8[:, t, :], ps, AF.Identity)
                        nc.vector.tensor_tensor(out=sq16[:, t, :],
                                                in0=qkt8[:, t, :],
                                                in1=qkt8[:, t, :], op=OP.mult)
                st_, sp_ = (p == 0), (p == NTH // 2 - 1)
                nc.tensor.matmul(ps_nrm, ones8, sq16[:, 2 * p, :],
                                 start=st_, stop=False)
                nc.tensor.matmul(ps_nrm, ones8, sq16[:, 2 * p + 1, :],
                                 start=False, stop=sp_)
                nc.tensor.matmul(ps_a0, qkt8[:, 2 * p:2 * p + 2, 0:128],
                                 qkt8[:, 2 * p:2 * p + 2, C:2 * C],
                                 start=st_, stop=sp_, perf_mode=DR)
                nc.tensor.matmul(ps_a1, qkt8[:, 2 * p:2 * p + 2, 128:C],
                                 qkt8[:, 2 * p:2 * p + 2, C:2 * C],
                                 start=st_, stop=sp_, perf_mode=DR)

            dump("d_qkt", qkt8[:, 0, :])

            # pack the used per-head diagonal 32x32 gram blocks + norms
            ca_tx = stg.tile([128, CCW], F32, tag="ca_tx")
            for hh in range(H_CH):
                ct, r0 = hh // 4, (hh % 4) * HD
                src_ps = ps_a0 if ct == 0 else ps_a1
                nc.vector.tensor_copy(ca_tx[r0:r0 + HD, ct * HD:(ct + 1) * HD],
                                      src_ps[r0:r0 + HD, hh * HD:(hh + 1) * HD])
            nrm_sb = stg.tile([1, 2 * C], F32, tag="nrm_sb")
            nc.vector.tensor_copy(nrm_sb, ps_nrm)
            ps_fl = pmm.tile([128, 2 * CT], F32, tag="mm", name="ps_fl")
            for i in range(2 * CT):
                nc.tensor.matmul(ps_fl[:, i:i + 1],
                                 nrm_sb[0:1, i * 128:(i + 1) * 128],
                                 ident32[0:1, 0:1], is_transpose=True,
                                 start=(i == 0), stop=(i == 2 * CT - 1))
            nc.vector.tensor_copy(ca_tx[:, 2 * HD:CCW], ps_fl)
            nc.sync.dma_start(out=cc_in[:, :], in_=ca_tx[:, :])
            nc.gpsimd.collective_compute(
                "AllGather", OP.bypass, replica_groups=REPLICA_GROUPS,
                ins=[cc_in[:, :]], outs=[cc_out[:, :, :]])

            # MLP2 weights arrive during the collective
            w3 = wload("w3_8", CT, DFF, tag="wmlp_a")
            w4 = wload("w4_8", FT, C, tag="wmlp_b")

            # v channel-major (overlaps the collective)
            vcm3 = ap.tile([128, CT, T], F8, tag="vcm")
            for ch in range(4):
                for ct in range(CT):
                    ps = pmm.tile([128, 512], F32, tag="mm")
                    for kt in range(CT):
                        nc.tensor.matmul(ps, vvw[:, kt, ct * 128:(ct + 1) * 128],
                                         n3cm[:, ch * 4:(ch + 1) * 4, kt, :],
                                         start=(kt == 0), stop=(kt == CT - 1))
                    with nc.allow_low_precision(reason="fp8 v"):
                        nc.scalar.activation(vcm3[:, ct, ch * 512:(ch + 1) * 512],
                                             ps, AF.Identity)


            # ---- post-collective epilogue ----
            ca_rx = stg.tile([128, 2, CCW], F32, tag="ca_rx")
            nc.sync.dma_start(out=ca_rx, in_=cc_out[:, :, :].rearrange("r p w -> p r w"))
            ca_tot = stg.tile([128, CCW], F32, tag="ca_tot")
            nc.vector.tensor_tensor(out=ca_tot, in0=ca_rx[:, 0, :],
                                    in1=ca_rx[:, 1, :], op=OP.add)

            dump("d_catot", ca_tot)
            nktot = ca_tot[:, 2 * HD:CCW]    # [128, 4]: qsumsq-cols | ksumsq-cols
            # inv norms via Newton rsqrt (columns, f32)
            invn = stg.tile([128, 2 * CT], F32, tag="invn")
            nw_t = tp.tile([128, 2 * CT], F32, tag="nw_t")
            nc.vector.tensor_scalar(out=invn.bitcast(I32), in0=nktot.bitcast(I32),
                                    scalar1=1, scalar2=None,
                                    op0=OP.logical_shift_right)
            nc.vector.tensor_scalar(out=invn.bitcast(I32), in0=invn.bitcast(I32),
                                    scalar1=-1, scalar2=MAGIC,
                                    op0=OP.mult, op1=OP.add)
            for _ in range(2):
                nc.vector.tensor_tensor(out=nw_t, in0=invn, in1=invn, op=OP.mult)
                nc.vector.tensor_tensor(out=nw_t, in0=nw_t, in1=nktot, op=OP.mult)
                nc.vector.tensor_scalar(out=nw_t, in0=nw_t, scalar1=-0.5,
                                        scalar2=1.5, op0=OP.mult, op1=OP.add)
                nc.vector.tensor_tensor(out=invn, in0=invn, in1=nw_t, op=OP.mult)
            # scale invq by temperature and the poly-exp 1/4 folding
            invq = stg.tile([128, CT], F32, tag="invq")
            nc.vector.tensor_tensor(out=invq, in0=invn[:, 0:CT], in1=temp_sb,
                                    op=OP.mult)
            nc.vector.tensor_scalar(out=invq, in0=invq, scalar1=0.25,
                                    scalar2=None, op0=OP.mult)
            # invk back to a row [1, C] via PE transpose, broadcast to [128, C]
            ps_kf = pmm.tile([1, C], F32, tag="mm", name="ps_kf")
            for ct in range(CT):
                nc.tensor.matmul(ps_kf[0:1, ct * 128:(ct + 1) * 128],
                                 invn[:, CT + ct:CT + ct + 1],
                                 ident32, is_transpose=True,
                                 start=(ct == 0), stop=(ct == 1))
            invk_row = tp.tile([1, C], F32, tag="invk_row")
            nc.vector.tensor_copy(invk_row, ps_kf)
            ones_row16 = tp.tile([1, 128], BF16, tag="ones_row16")
            nc.vector.memset(ones_row16, 1.0)
            invk_row16 = tp.tile([1, C], BF16, tag="invk_row16")
            with nc.allow_low_precision(reason="bf16 bcast operand"):
                nc.vector.tensor_copy(invk_row16, invk_row)
            ps_bk = pmm.tile([128, C], F32, tag="mm", name="ps_bk")
            nc.tensor.matmul(ps_bk, ones_row16, invk_row16, start=True, stop=True)
            bk = stg.tile([128, C], F32, tag="bk")
            nc.vector.tensor_copy(bk, ps_bk)

            # logits = gram * invq(part) * invk(elem); per-head blocks only
            attn_l = stg.tile([128, CT, 128], F32, tag="attn_l")
            nc.vector.memset(attn_l, 0.0)
            for hh in range(H_CH):
                ct, r0 = hh // 4, (hh % 4) * HD
                nc.vector.scalar_tensor_tensor(
                    out=attn_l[r0:r0 + HD, ct, r0:r0 + HD],
                    in0=ca_tot[r0:r0 + HD, ct * HD:(ct + 1) * HD],
                    scalar=invq[r0:r0 + HD, ct:ct + 1],
                    in1=bk[r0:r0 + HD, hh * HD:(hh + 1) * HD],
                    op0=OP.mult, op1=OP.mult)

            # exp via (1 + u + u^2/2 + u^3/6)^4; u = logits/4 folded into invq
            u = attn_l
            pe_ = stg.tile([128, CT, 128], F32, tag="attn_p")
            nc.vector.tensor_scalar(out=pe_, in0=u, scalar1=1.0 / 6.0,
                                    scalar2=0.5, op0=OP.mult, op1=OP.add)
            nc.vector.tensor_tensor(out=pe_, in0=pe_, in1=u, op=OP.mult)
            nc.vector.tensor_scalar(out=pe_, in0=pe_, scalar1=1.0, scalar2=None,
                                    op0=OP.add)
            nc.vector.tensor_tensor(out=pe_, in0=pe_, in1=u, op=OP.mult)
            nc.vector.tensor_scalar(out=pe_, in0=pe_, scalar1=1.0, scalar2=None,
                                    op0=OP.add)
            nc.vector.tensor_tensor(out=pe_, in0=pe_, in1=pe_, op=OP.mult)
            nc.vector.tensor_tensor(out=pe_, in0=pe_, in1=pe_, op=OP.mult)
            # mask off-block entries, row-normalize
            with nc.allow_low_precision(reason="bf16 mask"):
                nc.vector.tensor_tensor(out=pe_, in0=pe_, in1=bdm, op=OP.mult)
            rsum = tp.tile([128, CT], F32, tag="attn_rs")
            nc.vector.tensor_reduce(rsum.unsqueeze(-1), pe_, axis=AX.X, op=OP.add)
            rinv = tp.tile([128, CT], F32, tag="attn_ri")
            nc.vector.reciprocal(rinv, rsum)
            attn16 = stg.tile([128, CT, 128], BF16, tag="attn16")
            with nc.allow_low_precision(reason="bf16 attn"):
                nc.vector.tensor_tensor(out=attn16, in0=pe_,
                                        in1=bscale(rinv, 0, CT, 128), op=OP.mult)

            dump("d_attn", attn16)

            # transpose attn blocks -> lhsT; attn @ v ; proj ; add3
            at_bd = stg.tile([128, CT, 128], BF16, tag="at_bd")
            for ct in range(CT):
                ps_at = pmmt.tile([128, 128], BF16, tag="mmt", name=f"ps_at{ct}")
                nc.tensor.transpose(ps_at, attn16[:, ct, :], ident16)
                with nc.allow_low_precision(reason="bf16 attn lhsT"):
                    nc.vector.tensor_copy(at_bd[:, ct, :], ps_at)

            cac = ap.tile([128, CT, T], F8, tag="cac")
            for ct in range(CT):
                for ch in range(4):
                    ps = pmm.tile([128, 512], F32, tag="mm")
                    nc.tensor.matmul(ps, at_bd[:, ct, :],
                                     vcm3[:, ct, ch * 512:(ch + 1) * 512],
                                     start=True, stop=True)
                    with nc.allow_low_precision(reason="fp8 cac"):
                        nc.scalar.activation(cac[:, ct, ch * 512:(ch + 1) * 512],
                                             ps, AF.Identity)

            add3 = ap.tile([128, NTH, C], F32, tag="residA")
            rs4 = stg.tile([128, NTH], F32, tag="rs_ln4")
            x4n16, x4cm = cm_tiles("f")
            for G in range(4):
                for sg in range(2):
                    g = G * 2 + sg
                    ps = pmm.tile([128, 2, C], F32, tag="mm")
                    for j in range(2):
                        t = g * 2 + j
                        nc.tensor.matmul(ps[:, j, :], cac[:, :, t * 128:(t + 1) * 128],
                                         pjw, start=(j == 0), stop=(j == 1),
                                         perf_mode=DR)
                    nc.vector.scalar_tensor_tensor(
                        out=add3[:, g * 2:g * 2 + 2, :], in0=ps,
                        scalar=1.0 / (SW * SW), in1=add2[:, g * 2:g * 2 + 2, :],
                        op0=OP.mult, op1=OP.add)
                ln_chunk(add3, list(range(NTH)), rs4, G * 4, 4, "ln4")
                cm_group(add3, rs4, x4n16, x4cm, G, "f")

            # ================= MLP 2 (writes y) =================
            mlp(add3, x4cm, w3, w4, None, True)

    nc.compile()
    _CACHE["nc"] = nc
    return nc


def prep_host(inputs):
    """Weight/layout prep shared by all cores (no arithmetic on x)."""
    f8 = ml_dtypes.float8_e4m3
    f = lambda k: np.asarray(inputs[k], np.float32)
    for k in ("ln1_b", "ln2_b", "ln3_b", "ln4_b", "m1_b1", "m1_b2",
              "m2_b1", "m2_b2", "proj_b"):
        assert np.abs(f(k)).max() == 0.0, f"{k} nonzero; bias path not emitted"
    g1, g2, g3, g4 = f("ln1_g"), f("ln2_g"), f("ln3_g"), f("ln4_g")

    def fold(w, g, center=True):
        """M = Pc @ (diag(g) @ w.T): [in, out] with LN gamma + centering."""
        m = (w * g[None, :]).T.astype(np.float32)
        if center:
            m = m - m.mean(axis=0, keepdims=True)
        return m

    def arr_ct(m, scale):
        """[C_in, O] -> [128, C_in//128, O] fp8 with scale."""
        ci, o = m.shape
        return np.ascontiguousarray(
            (m * scale).reshape(ci // 128, 128, o).transpose(1, 0, 2)
        ).astype(f8)

    qkv_w = f("qkv_w")
    wkq = np.concatenate([fold(f("wk"), g1), fold(f("wq"), g1)], axis=1)
    qk_ca = fold(qkv_w[:2 * C], g3)          # [C, 2C]
    vv_ca = fold(qkv_w[2 * C:], g3)          # [C, C]
    w1 = fold(f("m1_w1"), g2)
    w3 = fold(f("m2_w1"), g4)
    w2 = f("m1_w2").T                        # [DFF, C]
    w4 = f("m2_w2").T
    wr = f("wr").T                           # [C, C]
    pj = f("proj_w").T

    bdmask = np.zeros((128, CT, 128), np.float32)
    for hh in range(H_CH):
        ct, r0 = hh // 4, (hh % 4) * HD
        bdmask[r0:r0 + HD, ct, r0:r0 + HD] = 1.0

    temp = np.repeat(f("temperature").reshape(H_CH), HD).reshape(CT, 128).T

    return {
        "wkq8": arr_ct(wkq, SW),
        "wv8": arr_ct(fold(f("wv"), g1), SW),
        "wr16": np.ascontiguousarray(
            (wr * SWR).reshape(CT, 128, C).transpose(1, 0, 2)
        ).astype(ml_dtypes.bfloat16),
        "qk8": arr_ct(qk_ca, SQK),
        "vv8": arr_ct(vv_ca, SW),
        "pj8": arr_ct(pj, SW),
        "w1_8": arr_ct(w1, SW),
        "w2_8": arr_ct(w2, SW),
        "w3_8": arr_ct(w3, SW),
        "w4_8": arr_ct(w4, SW),
        "ident8": np.eye(128, dtype=f8),
        "ident16": np.eye(128, dtype=ml_dtypes.bfloat16),
        "ones8": np.ones((128, 1), f8),
        "bdmask": bdmask.astype(ml_dtypes.bfloat16),
        "temp_c": np.ascontiguousarray(temp, np.float32),
        "ident32": np.eye(128, dtype=np.float32),
    }


def make_in_maps(inputs):
    shared = prep_host(inputs)
    f8 = ml_dtypes.float8_e4m3
    x = np.asarray(inputs["x"], np.float32)
    in_maps = []
    for c in range(NCORES):
        b, hlf = c // 2, c % 2
        # own half first, peer half second (layout only, no arithmetic)
        xo = np.concatenate([x[b, hlf * T:(hlf + 1) * T, :],
                             x[b, (1 - hlf) * T:(2 - hlf) * T, :]], axis=0)
        m = dict(shared)
        m["xt16"] = np.ascontiguousarray(
            xo.reshape(NT, 128, C).transpose(1, 0, 2)).astype(ml_dtypes.bfloat16)
        m["xcm8"] = np.ascontiguousarray(
            xo.T.reshape(CT, 128, N).transpose(1, 0, 2)).astype(f8)
        in_maps.append(m)
    return in_maps


def assemble(results):
    y = np.empty((B, N, C), np.float32)
    for c in range(NCORES):
        b, hlf = c // 2, c % 2
        y[b, hlf * T:(hlf + 1) * T, :] = results[c]["y"]
    return y


def kernel(**inputs):
    from concourse.bass_utils import run_bass_kernel_spmd

    nc = build_program()
    in_maps = make_in_maps(inputs)
    res = run_bass_kernel_spmd(nc, in_maps, list(range(NCORES)))
    return assemble(res.results)
